# revision 15
# baseline (speedup 1.0000x reference)
"""Bass/Trainium2 kernel for nn_BivariateSpectral: batched smallest-eigenvalue of
S_b = sym(A + B*diag(x_b) + C*diag(y_b)), b = 0..32767, each 64x64, 8 NeuronCores.

v2 (per core, data-parallel over batch):
  Phase 1 - batched Lanczos (K steps) on D_b = (M_b + M_b^T)/64 = S_b/32.
    D v = Ah v + Bh(x*v) + Ch(y*v) + x*(Bt v) + y*(Ct v); dim on partitions
    (two batch-halves as partitions 0-63/64-127, block-diag stationaries),
    batch on free dim.  PSUM is drained by the Scalar engine (activation
    copies) so the elementwise chain runs SBUF-only via scalar_tensor_tensor
    (2x_2p DVE mode).  alpha/beta^2 per column via ones-block-diag matmuls;
    beta-term folded into c before the alpha inner product (orthogonality).
    Tridiagonal rows staged to SBUF partitions 2j+h (no DRAM roundtrip).
  Phase 2 - PE-transpose the [2K, cols] stages into batch-on-partitions
    layout, then Sturm bisection (NS shifts x PASSES, division-free fp32
    char-poly recurrence).  Output scaled back by 32.
"""

import functools
import numpy as np

BATCH, DIM = 32768, 64
NCORES = 8
SHARD = BATCH // NCORES      # 4096 batch elems per core
NFREE = SHARD // 2           # 2048 free columns (two partition-halves)
CHUNK = 512                  # columns per group
NCH = NFREE // CHUNK         # 4 groups
K = 32                       # Lanczos steps
NB = K - 1
ROWS_A = 2 * K               # 64 rows in stage_a (2j+h)
ROWS_B = 2 * NB              # 62 rows in stage_b
TG = NFREE // 128            # 16 transpose column-groups
NS = 6                       # bisection shifts per pass
PASSES = 3
C_OP = np.float32(1.0 / 64.0)   # A,B,C host prescale: D = (M+M^T)/64 = S/32
OUT_SCALE = 16.0                # lam_S = 32 * 0.5 * (lo+hi)


def _v0_vec():
    rng = np.random.default_rng(1234)
    v = rng.standard_normal(DIM).astype(np.float64)
    v /= np.sqrt((v * v).sum())
    return v.astype(np.float32)


def _bd(m):
    out = np.zeros((128, 128), np.float32)
    out[:64, :64] = m
    out[64:, 64:] = m
    return out


def _bcast_s(ap, extra_off=0, count=2, ns=None):
    """[128, T, R] AP -> [128, ns, T, count] with 0-step shift dim."""
    import concourse.bass as bass
    if ns is None:
        ns = NS
    dims = list(ap.ap)
    part = dims[0]
    tdim = dims[1]
    return bass.AP(
        tensor=ap.tensor,
        offset=ap.offset + extra_off,
        ap=[part, [0, ns], tdim, [1, count]],
    )


def _bcast_flat(ap, ns=None):
    """[128, T, 2] AP -> [128, ns, T, 2] via 0-step shift dim."""
    import concourse.bass as bass
    if ns is None:
        ns = NS
    dims = list(ap.ap)
    return bass.AP(tensor=ap.tensor, offset=ap.offset, ap=[dims[0], [0, ns]] + dims[1:])


def _two_rows(ap, stride=64, count=2):
    """AP over a [128, F] tile selecting partitions {0, stride}: -> [2, F]."""
    import concourse.bass as bass
    dims = list(ap.ap)
    return bass.AP(
        tensor=ap.tensor, offset=ap.offset, ap=[[stride, count]] + dims[1:]
    )


def _flat(ap):
    """Contiguous tile AP -> 2-dim [128, total_free] AP."""
    import concourse.bass as bass
    import numpy as np
    dims = list(ap.ap)
    total = 1
    for st, ct in dims[1:]:
        total *= ct
    return bass.AP(tensor=ap.tensor, offset=ap.offset,
                   ap=[dims[0], [1, total]])


def _brow(ap, j, seg, ns=None):
    """j-major flat tile AP -> [128, ns(bcast), seg] at row j."""
    import concourse.bass as bass
    if ns is None:
        ns = NS
    dims = list(ap.ap)
    return bass.AP(tensor=ap.tensor, offset=ap.offset + j * seg,
                   ap=[dims[0], [0, ns], [1, seg]])


def _jmajor(ap, nj, nt, step0=2, inner_stride=None):
    """[128, T, R] t-major AP -> 4-dim [128, nj, T, 2] j-major read AP."""
    import concourse.bass as bass
    dims = list(ap.ap)
    rstride = dims[2][0] if len(dims) > 2 else 1
    tstride = dims[1][0]
    return bass.AP(tensor=ap.tensor, offset=ap.offset,
                   ap=[dims[0], [step0 * rstride, nj], [tstride, nt],
                       [rstride, 2]])


def _strided_free(ap, stride, count, inner):
    """[128, F] tile AP -> [128, count, inner] with free stride for count dim."""
    import concourse.bass as bass
    dims = list(ap.ap)
    return bass.AP(
        tensor=ap.tensor, offset=ap.offset,
        ap=[dims[0], [stride, count], [1, inner]],
    )


@functools.lru_cache(maxsize=4)
def _program(idx: int):
    import concourse.bacc as bacc
    import concourse.bass as bass
    import concourse.mybir as mybir
    import concourse.tile as tile
    from concourse.masks import make_identity

    F32 = mybir.dt.float32
    F32R = mybir.dt.float32r
    BF16 = mybir.dt.bfloat16
    I32 = mybir.dt.int32
    OP = mybir.AluOpType
    ACTF = mybir.ActivationFunctionType

    nc = bacc.Bacc("TRN2", target_bir_lowering=False, debug=False)

    x_in = nc.dram_tensor("x", [128, NFREE], F32, kind="ExternalInput").ap()
    y_in = nc.dram_tensor("y", [128, NFREE], F32, kind="ExternalInput").ap()
    lms_in = nc.dram_tensor("lms", [128, 128], BF16, kind="ExternalInput").ap()
    lbf_in = nc.dram_tensor("lbf", [128, 128], BF16, kind="ExternalInput").ap()
    lcf_in = nc.dram_tensor("lcf", [128, 128], BF16, kind="ExternalInput").ap()
    lbt_in = nc.dram_tensor("lbt", [128, 128], BF16, kind="ExternalInput").ap()
    lct_in = nc.dram_tensor("lct", [128, 128], BF16, kind="ExternalInput").ap()
    obd_in = nc.dram_tensor("obd", [128, 128], BF16, kind="ExternalInput").ap()
    v0_in = nc.dram_tensor("v0", [128, 1], F32, kind="ExternalInput").ap()
    lam_out = nc.dram_tensor("lam", [SHARD], F32, kind="ExternalOutput").ap()

    def stt(out, in0, in1, op1, scalar=1.0, op0=OP.mult):
        nc.vector.scalar_tensor_tensor(out=out, in0=in0, scalar=scalar,
                                       in1=in1, op0=op0, op1=op1)

    with tile.TileContext(nc) as tc:
        with tc.tile_pool(name="stage", bufs=1) as stagep:
            stage_a = stagep.tile([128, NFREE], F32)
            stage_b = stagep.tile([128, NFREE], F32)

            # ---------------- Phase 1: Lanczos ----------------
            with (
                tc.tile_pool(name="singles", bufs=1) as singles,
                tc.tile_pool(name="vpool", bufs=3) as vpool,
                tc.tile_pool(name="work", bufs=1) as work,
                tc.tile_pool(name="bbp", bufs=2) as bbp,
                tc.tile_pool(name="p3p", bufs=2, space="PSUM") as p3p,
                tc.tile_pool(name="p4p", bufs=2, space="PSUM") as p4p,
                tc.tile_pool(name="pwp", bufs=2, space="PSUM") as pwp,
                tc.tile_pool(name="pbc", bufs=2, space="PSUM") as pbcp,
            ):
                xt = singles.tile([128, NFREE], F32)
                yt = singles.tile([128, NFREE], F32)
                nc.sync.dma_start(out=xt[:], in_=x_in)
                nc.sync.dma_start(out=yt[:], in_=y_in)
                xtb = singles.tile([128, NFREE], BF16)
                ytb = singles.tile([128, NFREE], BF16)
                nc.vector.tensor_copy(xtb[:], xt[:])
                nc.vector.tensor_copy(ytb[:], yt[:])
                mats = {}
                for nm, src in (("lms", lms_in), ("lbf", lbf_in),
                                ("lcf", lcf_in), ("lbt", lbt_in),
                                ("lct", lct_in), ("obd", obd_in)):
                    t_ = singles.tile([128, 128], BF16, name=f"m_{nm}")
                    nc.sync.dma_start(out=t_[:], in_=src)
                    mats[nm] = t_
                lms_r, lbf_r, lcf_r = mats["lms"], mats["lbf"], mats["lcf"]
                lbt_r, lct_r, obd_r = mats["lbt"], mats["lct"], mats["obd"]
                identt = singles.tile([128, 128], F32)
                make_identity(nc, identt[:])
                ident_r = singles.tile([128, 128], BF16)
                nc.vector.tensor_copy(ident_r[:], identt[:])
                identn_r = singles.tile([128, 128], BF16)
                nc.scalar.activation(identn_r[:], identt[:], ACTF.Copy,
                                     scale=-1.0)
                v0t = singles.tile([128, 1], F32)
                nc.sync.dma_start(out=v0t[:], in_=v0_in)
                epst = singles.tile([128, 1], F32)
                nc.vector.memset(epst[:], 1e-12)

                st = []
                for g in range(NCH):
                    v_cur = vpool.tile([128, CHUNK], BF16, tag=f"v{g}")
                    nc.vector.tensor_copy(
                        v_cur[:], v0t[:, 0:1].to_broadcast((128, CHUNK)))
                    st.append({"v": v_cur, "vp": None, "bb": None})

                for j in range(K):
                    last = j == K - 1
                    T = [{} for _ in range(NCH)]
                    # --- front muls, lock-step: t4 (earliest deps), t1/t2 ---
                    for g in range(NCH):
                        S, D = st[g], T[g]
                        if j > 0 and not last:
                            D["t4"] = work.tile([128, CHUNK], BF16,
                                                tag=f"t4{g}", name=f"t4{g}")
                            nc.gpsimd.tensor_mul(D["t4"][:], S["bb"][:],
                                                 S["vp"][:])
                    for g in range(NCH):
                        S, D = st[g], T[g]
                        gsl = slice(g * CHUNK, (g + 1) * CHUNK)
                        D["t1"] = work.tile([128, CHUNK], BF16, tag=f"t1{g}",
                                            name=f"t1{g}")
                        D["t2"] = work.tile([128, CHUNK], BF16, tag=f"t2{g}",
                                            name=f"t2{g}")
                        nc.gpsimd.tensor_mul(D["t1"][:], xtb[:, gsl],
                                             S["v"][:])
                        nc.vector.tensor_mul(D["t2"][:], ytb[:, gsl],
                                             S["v"][:])
                    # --- fused per-group chain ---
                    for g in range(NCH):
                        S, D = st[g], T[g]
                        v_cur = S["v"]
                        gsl = slice(g * CHUNK, (g + 1) * CHUNK)
                        csl = gsl
                        has_t4 = j > 0 and not last
                        p3 = p3p.tile([128, CHUNK], F32, tag="p3")
                        p4 = p4p.tile([128, CHUNK], F32, tag="p4")
                        pw = pwp.tile([128, CHUNK], F32, tag="pw")
                        nc.tensor.matmul(p3[:], lbt_r[:], v_cur[:],
                                         start=True, stop=True)
                        nc.tensor.matmul(p4[:], lct_r[:], v_cur[:],
                                         start=True, stop=True)
                        nc.tensor.matmul(pw[:], lms_r[:], v_cur[:],
                                         start=True, stop=False)
                        nc.tensor.matmul(pw[:], lbf_r[:], D["t1"][:],
                                         start=False, stop=False)
                        nc.tensor.matmul(pw[:], lcf_r[:], D["t2"][:],
                                         start=False, stop=False)
                        p3s = work.tile([128, CHUNK], BF16, tag=f"p3s{g}",
                                        name=f"p3s{g}")
                        p4s = work.tile([128, CHUNK], BF16, tag=f"p4s{g}",
                                        name=f"p4s{g}")
                        nc.scalar.activation(p3s[:], p3[:], ACTF.Copy)
                        nc.scalar.activation(p4s[:], p4[:], ACTF.Copy)
                        m1 = work.tile([128, CHUNK], BF16, tag=f"m1{g}")
                        m2 = work.tile([128, CHUNK], BF16, tag=f"m2{g}")
                        nc.gpsimd.tensor_mul(m1[:], xtb[:, gsl], p3s[:])
                        nc.vector.tensor_mul(m2[:], ytb[:, gsl], p4s[:])
                        nc.tensor.matmul(pw[:], ident_r[:], m1[:],
                                         start=False, stop=False)
                        nc.tensor.matmul(pw[:], ident_r[:], m2[:],
                                         start=False, stop=not has_t4)
                        if has_t4:
                            nc.tensor.matmul(pw[:], identn_r[:], D["t4"][:],
                                             start=False, stop=True)
                        cs_t = work.tile([128, CHUNK], F32, tag=f"cs{g}",
                                         name=f"cs{g}")
                        nc.scalar.activation(cs_t[:], pw[:], ACTF.Copy)
                        p_t = work.tile([128, CHUNK], BF16, tag=f"pq{g}")
                        nc.vector.tensor_mul(p_t[:], v_cur[:], cs_t[:])
                        ab = pbcp.tile([128, CHUNK], F32, tag="pbc")
                        nc.tensor.matmul(ab[:], obd_r[:], p_t[:],
                                         start=True, stop=True)
                        ra0 = work.tile([1, CHUNK], F32, tag=f"ra0{g}",
                                        name=f"ra0{g}")
                        ra1 = work.tile([1, CHUNK], F32, tag=f"ra1{g}",
                                        name=f"ra1{g}")
                        nc.scalar.activation(ra0[:], ab[0:1, :], ACTF.Copy)
                        nc.scalar.activation(ra1[:], ab[64:65, :], ACTF.Copy)
                        nc.sync.dma_start(out=stage_a[2 * j : 2 * j + 1, csl],
                                          in_=ra0[:])
                        nc.sync.dma_start(
                            out=stage_a[2 * j + 1 : 2 * j + 2, csl],
                            in_=ra1[:])
                        if not last:
                            t3 = work.tile([128, CHUNK], F32, tag=f"m1{g}")
                            nc.vector.tensor_mul(t3[:], ab[:], v_cur[:])
                            w = work.tile([128, CHUNK], F32, tag=f"m2{g}")
                            nc.vector.tensor_sub(w[:], cs_t[:], t3[:])
                            q_t = work.tile([128, CHUNK], BF16, tag=f"pq{g}")
                            nc.scalar.activation(q_t[:], w[:], ACTF.Square)
                            b2 = p3p.tile([128, CHUNK], F32, tag="p3")
                            nc.tensor.matmul(b2[:], obd_r[:], q_t[:],
                                             start=True, stop=True)
                            bb = bbp.tile([128, CHUNK], F32, tag=f"bb{g}")
                            nc.scalar.activation(bb[:], b2[:], ACTF.Sqrt,
                                                 bias=epst[:], scale=1.0)
                            nc.sync.dma_start(
                                out=stage_b[2 * j : 2 * j + 1, csl],
                                in_=bb[0:1, :])
                            nc.sync.dma_start(
                                out=stage_b[2 * j + 1 : 2 * j + 2, csl],
                                in_=bb[64:65, :])
                            rb = work.tile([128, CHUNK], F32, tag=f"p3s{g}")
                            nc.vector.reciprocal_approx_fast(out=rb[:],
                                                             in_=bb[:])
                            v_nxt = vpool.tile([128, CHUNK], BF16,
                                               tag=f"v{g}")
                            nc.vector.tensor_mul(v_nxt[:], w[:], rb[:])
                            S["vp"] = S["v"]
                            S["v"] = v_nxt
                            S["bb"] = bb

            # ---------------- Phase 2: transpose + Sturm ----------------
            with (
                tc.tile_pool(name="bis", bufs=1) as bis,
                tc.tile_pool(name="st3", bufs=1) as st3,
                tc.tile_pool(name="ptp", bufs=1, space="PSUM") as ptp,
            ):
                ident = bis.tile([128, 128], F32)
                make_identity(nc, ident[:])

                pta = ptp.tile([128, TG * ROWS_A], F32, tag="pta")
                ptb = ptp.tile([128, TG * 64], F32, tag="ptb")
                for t in range(TG):
                    csl = slice(t * 128, (t + 1) * 128)
                    nc.tensor.transpose(pta[:, t * ROWS_A : (t + 1) * ROWS_A],
                                        stage_a[0:ROWS_A, csl],
                                        ident[0:ROWS_A, 0:ROWS_A])
                    nc.tensor.transpose(
                        ptb[:, t * 64 : t * 64 + ROWS_B],
                        stage_b[0:ROWS_B, csl], ident[0:ROWS_B, 0:ROWS_B])
                td_a = bis.tile([128, TG, ROWS_A], F32)
                td_b = bis.tile([128, TG, ROWS_B], F32)
                nc.vector.tensor_copy(td_a[:], pta[:])
                nc.vector.tensor_copy(td_b[:],
                                      _strided_free(ptb[:], 64, TG, ROWS_B))

                import concourse.bass as bass_mod

                def jdims_ap(tile_ap, nj, step0=2):
                    d = list(tile_ap.ap)
                    return bass_mod.AP(
                        tensor=tile_ap.tensor, offset=tile_ap.offset,
                        ap=[d[0], d[1], [1, 2], [step0, nj]],
                    )

                absb = td_b
                g = bis.tile([128, TG, ROWS_A], F32)
                nc.vector.tensor_copy(g[:], td_a[:])
                stt(g[:, :, 2:ROWS_A], g[:, :, 2:ROWS_A], absb[:],
                    OP.subtract)
                stt(g[:, :, 0:ROWS_B], g[:, :, 0:ROWS_B], absb[:],
                    OP.subtract)

                lo = bis.tile([128, TG, 2], F32)
                hi = bis.tile([128, TG, 2], F32)
                nc.vector.tensor_reduce(lo[:], jdims_ap(g[:], K),
                                        mybir.AxisListType.X, OP.min)
                if idx == 0:
                    nc.vector.tensor_reduce(hi[:], jdims_ap(td_a[:], K),
                                            mybir.AxisListType.X, OP.min)
                else:
                    g2 = g
                    nc.vector.tensor_copy(g2[:], td_a[:])
                    stt(g2[:, :, 2:ROWS_A], g2[:, :, 2:ROWS_A], absb[:],
                        OP.add)
                    stt(g2[:, :, 0:ROWS_B], g2[:, :, 0:ROWS_B], absb[:],
                        OP.add)
                    nc.vector.tensor_reduce(hi[:], jdims_ap(g2[:], K),
                                            mybir.AxisListType.X, OP.max)

                # j-major flat copies of the tridiagonal (stt needs <=3-dim APs)
                SEG = TG * 2
                td_a2 = bis.tile([128, K * SEG], F32)
                td_b2 = bis.tile([128, NB * SEG], F32)
                nc.vector.tensor_copy(td_a2[:], _jmajor(td_a[:], K, TG))
                nc.vector.tensor_copy(td_b2[:], _jmajor(td_b[:], NB, TG))
                stt(td_b2[:], td_b2[:], td_b2[:], OP.mult)

                cs = bis.tile([128, NS, TG, 2], F32)
                for s in range(NS):
                    nc.vector.memset(cs[:, s, :, :],
                                     float(s + 1) / float(NS + 1))

                sig = bis.tile([128, NS, TG, 2], F32)
                d_t = bis.tile([128, TG, 2], F32)
                pA = st3.tile([128, NS * SEG], F32, tag="pA")
                pB = st3.tile([128, NS * SEG], F32, tag="pB")
                pC = st3.tile([128, NS * SEG], F32, tag="pC")
                cA = st3.tile([128, NS, TG, 2], F32, tag="cA")
                cB = st3.tile([128, NS, TG, 2], F32, tag="cB")
                ca_t = st3.tile([128, NS * SEG], F32, tag="ca")
                u_t = st3.tile([128, NS * SEG], F32, tag="u")
                tb_t = st3.tile([128, NS * SEG], F32, tag="tb")
                sg_t = st3.tile([128, NS * SEG], F32, tag="sg")
                mle = bis.tile([128, TG, 2], I32)
                mgt = bis.tile([128, TG, 2], I32)

                thr = float(idx) + 0.5
                for ip in range(PASSES):
                    stt(d_t[:], hi[:], lo[:], OP.subtract)
                    stt(_flat(sig[:]), _flat(cs[:]),
                        _brow(_flat(d_t[:]), 0, SEG), OP.mult)
                    stt(_flat(sig[:]), _flat(sig[:]),
                        _brow(_flat(lo[:]), 0, SEG), OP.add)
                    po, pp, pn = pA, pB, pC
                    nc.vector.memset(po[:], 1.0)
                    stt(pp[:], _brow(td_a2[:], 0, SEG), _flat(sig[:]),
                        OP.subtract)
                    cnt, cnt_nxt = cA, cB
                    nc.vector.tensor_scalar(out=_flat(cnt[:]), in0=pp[:],
                                            scalar1=0.0, scalar2=None,
                                            op0=OP.is_lt)
                    for j in range(1, K):
                        stt(ca_t[:], _brow(td_a2[:], j, SEG), _flat(sig[:]),
                            OP.subtract)
                        stt(u_t[:], ca_t[:], pp[:], OP.mult)
                        stt(tb_t[:], _brow(td_b2[:], j - 1, SEG), po[:],
                            OP.mult)
                        stt(pn[:], u_t[:], tb_t[:], OP.subtract)
                        stt(sg_t[:], pn[:], pp[:], OP.mult)
                        nc.vector.scalar_tensor_tensor(
                            out=_flat(cnt_nxt[:]), in0=sg_t[:], scalar=0.0,
                            in1=_flat(cnt[:]), op0=OP.is_lt, op1=OP.add)
                        po, pp, pn = pp, pn, po
                        cnt, cnt_nxt = cnt_nxt, cnt
                    for s in range(NS):
                        nc.vector.tensor_scalar(out=mle[:],
                                                in0=cnt[:, s, :, :],
                                                scalar1=thr, scalar2=None,
                                                op0=OP.is_le)
                        nc.vector.copy_predicated(out=lo[:], mask=mle[:],
                                                  data=sig[:, s, :, :])
                    for s in range(NS - 1, -1, -1):
                        nc.vector.tensor_scalar(out=mgt[:],
                                                in0=cnt[:, s, :, :],
                                                scalar1=thr, scalar2=None,
                                                op0=OP.is_gt)
                        nc.vector.copy_predicated(out=hi[:], mask=mgt[:],
                                                  data=sig[:, s, :, :])

                lam_t = bis.tile([128, TG, 2], F32)
                stt(lam_t[:], lo[:], hi[:], OP.add)
                nc.vector.tensor_scalar(out=lam_t[:], in0=lam_t[:],
                                        scalar1=OUT_SCALE, scalar2=None,
                                        op0=OP.mult)
                lam_ap = lam_out.rearrange("(h t p) -> h p t", h=2, t=TG,
                                           p=128)
                for h in range(2):
                    nc.sync.dma_start(out=lam_ap[h], in_=lam_t[:, :, h])

    nc.compile()
    return nc


def kernel(x, y, A, B, C, eigval_idx):
    from concourse.bass_utils import run_bass_kernel_spmd

    idx = int(np.asarray(eigval_idx))
    nc = _program(idx)

    import ml_dtypes

    A32 = np.asarray(A, np.float32) * C_OP
    B32 = np.asarray(B, np.float32) * C_OP
    C32 = np.asarray(C, np.float32) * C_OP
    bf = ml_dtypes.bfloat16
    lms = _bd(A32 + A32.T).astype(bf)
    lbf = _bd(B32.T).astype(bf)
    lcf = _bd(C32.T).astype(bf)
    lbt = _bd(B32).astype(bf)
    lct = _bd(C32).astype(bf)
    obd = _bd(np.ones((64, 64), np.float32)).astype(bf)
    v0 = np.concatenate([_v0_vec(), _v0_vec()]).reshape(128, 1)

    xT = np.ascontiguousarray(np.asarray(x, np.float32).T)  # (64, BATCH)
    yT = np.ascontiguousarray(np.asarray(y, np.float32).T)

    in_maps = []
    for c in range(NCORES):
        b0 = c * SHARD
        xc = np.concatenate(
            [xT[:, b0 : b0 + NFREE], xT[:, b0 + NFREE : b0 + SHARD]], axis=0
        )
        yc = np.concatenate(
            [yT[:, b0 : b0 + NFREE], yT[:, b0 + NFREE : b0 + SHARD]], axis=0
        )
        in_maps.append(
            {
                "x": np.ascontiguousarray(xc),
                "y": np.ascontiguousarray(yc),
                "lms": lms, "lbf": lbf, "lcf": lcf, "lbt": lbt, "lct": lct,
                "obd": obd, "v0": v0,
            }
        )

    res = run_bass_kernel_spmd(nc, in_maps, core_ids=list(range(NCORES)))
    out = np.concatenate([res.results[c]["lam"] for c in range(NCORES)])
    return out.reshape(BATCH, 1).astype(np.float32)


# revision 16
# speedup vs baseline: 1.4340x; 1.4340x over previous
"""Bass/Trainium2 kernel for nn_BivariateSpectral: batched smallest-eigenvalue of
S_b = sym(A + B*diag(x_b) + C*diag(y_b)), b = 0..32767, each 64x64, 8 NeuronCores.

v2 (per core, data-parallel over batch):
  Phase 1 - batched Lanczos (K steps) on D_b = (M_b + M_b^T)/64 = S_b/32.
    D v = Ah v + Bh(x*v) + Ch(y*v) + x*(Bt v) + y*(Ct v); dim on partitions
    (two batch-halves as partitions 0-63/64-127, block-diag stationaries),
    batch on free dim.  PSUM is drained by the Scalar engine (activation
    copies) so the elementwise chain runs SBUF-only via scalar_tensor_tensor
    (2x_2p DVE mode).  alpha/beta^2 per column via ones-block-diag matmuls;
    beta-term folded into c before the alpha inner product (orthogonality).
    Tridiagonal rows staged to SBUF partitions 2j+h (no DRAM roundtrip).
  Phase 2 - PE-transpose the [2K, cols] stages into batch-on-partitions
    layout, then Sturm bisection (NS shifts x PASSES, division-free fp32
    char-poly recurrence).  Output scaled back by 32.
"""

import functools
import numpy as np

BATCH, DIM = 32768, 64
NCORES = 8
SHARD = BATCH // NCORES      # 4096 batch elems per core
NFREE = SHARD // 2           # 2048 free columns (two partition-halves)
CHUNK = 1024                 # columns per group
NCH = NFREE // CHUNK         # 2 groups
K = 32                       # Lanczos steps
NB = K - 1
ROWS_A = 2 * K               # 64 rows in stage_a (2j+h)
ROWS_B = 2 * NB              # 62 rows in stage_b
TG = NFREE // 128            # 16 transpose column-groups
NS = 6                       # bisection shifts per pass
PASSES = 3
C_OP = np.float32(1.0 / 64.0)   # A,B,C host prescale: D = (M+M^T)/64 = S/32
OUT_SCALE = 16.0                # lam_S = 32 * 0.5 * (lo+hi)


def _v0_vec():
    rng = np.random.default_rng(1234)
    v = rng.standard_normal(DIM).astype(np.float64)
    v /= np.sqrt((v * v).sum())
    return v.astype(np.float32)


def _bd(m):
    out = np.zeros((128, 128), np.float32)
    out[:64, :64] = m
    out[64:, 64:] = m
    return out


def _bcast_s(ap, extra_off=0, count=2, ns=None):
    """[128, T, R] AP -> [128, ns, T, count] with 0-step shift dim."""
    import concourse.bass as bass
    if ns is None:
        ns = NS
    dims = list(ap.ap)
    part = dims[0]
    tdim = dims[1]
    return bass.AP(
        tensor=ap.tensor,
        offset=ap.offset + extra_off,
        ap=[part, [0, ns], tdim, [1, count]],
    )


def _bcast_flat(ap, ns=None):
    """[128, T, 2] AP -> [128, ns, T, 2] via 0-step shift dim."""
    import concourse.bass as bass
    if ns is None:
        ns = NS
    dims = list(ap.ap)
    return bass.AP(tensor=ap.tensor, offset=ap.offset, ap=[dims[0], [0, ns]] + dims[1:])


def _two_rows(ap, stride=64, count=2):
    """AP over a [128, F] tile selecting partitions {0, stride}: -> [2, F]."""
    import concourse.bass as bass
    dims = list(ap.ap)
    return bass.AP(
        tensor=ap.tensor, offset=ap.offset, ap=[[stride, count]] + dims[1:]
    )


def _flat(ap):
    """Contiguous tile AP -> 2-dim [128, total_free] AP."""
    import concourse.bass as bass
    import numpy as np
    dims = list(ap.ap)
    total = 1
    for st, ct in dims[1:]:
        total *= ct
    return bass.AP(tensor=ap.tensor, offset=ap.offset,
                   ap=[dims[0], [1, total]])


def _brow(ap, j, seg, ns=None):
    """j-major flat tile AP -> [128, ns(bcast), seg] at row j."""
    import concourse.bass as bass
    if ns is None:
        ns = NS
    dims = list(ap.ap)
    return bass.AP(tensor=ap.tensor, offset=ap.offset + j * seg,
                   ap=[dims[0], [0, ns], [1, seg]])


def _jmajor(ap, nj, nt, step0=2, inner_stride=None):
    """[128, T, R] t-major AP -> 4-dim [128, nj, T, 2] j-major read AP."""
    import concourse.bass as bass
    dims = list(ap.ap)
    rstride = dims[2][0] if len(dims) > 2 else 1
    tstride = dims[1][0]
    return bass.AP(tensor=ap.tensor, offset=ap.offset,
                   ap=[dims[0], [step0 * rstride, nj], [tstride, nt],
                       [rstride, 2]])


def _strided_free(ap, stride, count, inner):
    """[128, F] tile AP -> [128, count, inner] with free stride for count dim."""
    import concourse.bass as bass
    dims = list(ap.ap)
    return bass.AP(
        tensor=ap.tensor, offset=ap.offset,
        ap=[dims[0], [stride, count], [1, inner]],
    )


@functools.lru_cache(maxsize=4)
def _program(idx: int):
    import concourse.bacc as bacc
    import concourse.bass as bass
    import concourse.mybir as mybir
    import concourse.tile as tile
    from concourse.masks import make_identity

    F32 = mybir.dt.float32
    F32R = mybir.dt.float32r
    BF16 = mybir.dt.bfloat16
    I32 = mybir.dt.int32
    OP = mybir.AluOpType
    ACTF = mybir.ActivationFunctionType

    nc = bacc.Bacc("TRN2", target_bir_lowering=False, debug=False)

    x_in = nc.dram_tensor("x", [128, NFREE], F32, kind="ExternalInput").ap()
    y_in = nc.dram_tensor("y", [128, NFREE], F32, kind="ExternalInput").ap()
    lms_in = nc.dram_tensor("lms", [128, 128], BF16, kind="ExternalInput").ap()
    lbf_in = nc.dram_tensor("lbf", [128, 128], BF16, kind="ExternalInput").ap()
    lcf_in = nc.dram_tensor("lcf", [128, 128], BF16, kind="ExternalInput").ap()
    lbt_in = nc.dram_tensor("lbt", [128, 128], BF16, kind="ExternalInput").ap()
    lct_in = nc.dram_tensor("lct", [128, 128], BF16, kind="ExternalInput").ap()
    obd_in = nc.dram_tensor("obd", [128, 128], BF16, kind="ExternalInput").ap()
    v0_in = nc.dram_tensor("v0", [128, 1], F32, kind="ExternalInput").ap()
    lam_out = nc.dram_tensor("lam", [SHARD], F32, kind="ExternalOutput").ap()

    def stt(out, in0, in1, op1, scalar=1.0, op0=OP.mult):
        nc.vector.scalar_tensor_tensor(out=out, in0=in0, scalar=scalar,
                                       in1=in1, op0=op0, op1=op1)

    with tile.TileContext(nc) as tc:
        with tc.tile_pool(name="stage", bufs=1) as stagep:
            stage_a = stagep.tile([128, NFREE], F32)
            stage_b = stagep.tile([128, NFREE], F32)

            # ---------------- Phase 1: Lanczos ----------------
            with (
                tc.tile_pool(name="singles", bufs=1) as singles,
                tc.tile_pool(name="vpool", bufs=3) as vpool,
                tc.tile_pool(name="work", bufs=1) as work,
                tc.tile_pool(name="bbp", bufs=2) as bbp,
                tc.tile_pool(name="p3p", bufs=1, space="PSUM") as p3p,
                tc.tile_pool(name="p4p", bufs=1, space="PSUM") as p4p,
                tc.tile_pool(name="pwp", bufs=1, space="PSUM") as pwp,
                tc.tile_pool(name="pbc", bufs=1, space="PSUM") as pbcp,
            ):
                xt = singles.tile([128, NFREE], F32)
                yt = singles.tile([128, NFREE], F32)
                nc.sync.dma_start(out=xt[:], in_=x_in)
                nc.sync.dma_start(out=yt[:], in_=y_in)
                xtb = singles.tile([128, NFREE], BF16)
                ytb = singles.tile([128, NFREE], BF16)
                nc.vector.tensor_copy(xtb[:], xt[:])
                nc.vector.tensor_copy(ytb[:], yt[:])
                mats = {}
                for nm, src in (("lms", lms_in), ("lbf", lbf_in),
                                ("lcf", lcf_in), ("lbt", lbt_in),
                                ("lct", lct_in), ("obd", obd_in)):
                    t_ = singles.tile([128, 128], BF16, name=f"m_{nm}")
                    nc.sync.dma_start(out=t_[:], in_=src)
                    mats[nm] = t_
                lms_r, lbf_r, lcf_r = mats["lms"], mats["lbf"], mats["lcf"]
                lbt_r, lct_r, obd_r = mats["lbt"], mats["lct"], mats["obd"]
                identt = singles.tile([128, 128], F32)
                make_identity(nc, identt[:])
                ident_r = singles.tile([128, 128], BF16)
                nc.vector.tensor_copy(ident_r[:], identt[:])
                identn_r = singles.tile([128, 128], BF16)
                nc.scalar.activation(identn_r[:], identt[:], ACTF.Copy,
                                     scale=-1.0)
                v0t = singles.tile([128, 1], F32)
                nc.sync.dma_start(out=v0t[:], in_=v0_in)
                epst = singles.tile([128, 1], F32)
                nc.vector.memset(epst[:], 1e-12)

                st = []
                for g in range(NCH):
                    v_cur = vpool.tile([128, CHUNK], BF16, tag=f"v{g}")
                    nc.vector.tensor_copy(
                        v_cur[:], v0t[:, 0:1].to_broadcast((128, CHUNK)))
                    st.append({"v": v_cur, "vp": None, "bb": None})

                for j in range(K):
                    last = j == K - 1
                    T = [{} for _ in range(NCH)]
                    # --- front muls: t4 first (earliest deps) then t1/t2 ---
                    for g in range(NCH):
                        S, D = st[g], T[g]
                        if j > 0 and not last:
                            D["t4"] = work.tile([128, CHUNK], BF16,
                                                tag=f"t4{g}", name=f"t4{g}")
                            nc.gpsimd.tensor_mul(D["t4"][:], S["bb"][:],
                                                 S["vp"][:])
                    for g in range(NCH):
                        S, D = st[g], T[g]
                        gsl = slice(g * CHUNK, (g + 1) * CHUNK)
                        D["t1"] = work.tile([128, CHUNK], BF16, tag=f"t1{g}", name=f"t1{g}")
                        D["t2"] = work.tile([128, CHUNK], BF16, tag=f"t2{g}", name=f"t2{g}")
                        nc.gpsimd.tensor_mul(D["t1"][:], xtb[:, gsl], S["v"][:])
                        nc.vector.tensor_mul(D["t2"][:], ytb[:, gsl], S["v"][:])
                    # --- matmuls, grouped by stationary within a group ---
                    for g in range(NCH):
                        S, D = st[g], T[g]
                        v_cur = S["v"]
                        p3 = p3p.tile([128, CHUNK], F32, tag="p3")
                        p4 = p4p.tile([128, CHUNK], F32, tag="p4")
                        pw = pwp.tile([128, CHUNK], F32, tag="pw")
                        D["p3"], D["p4"], D["pw"] = p3, p4, pw
                        for n0 in (0, 512):
                            ns = slice(n0, n0 + 512)
                            nc.tensor.matmul(p3[:, ns], lbt_r[:],
                                             v_cur[:, ns], start=True,
                                             stop=True)
                        for n0 in (0, 512):
                            ns = slice(n0, n0 + 512)
                            nc.tensor.matmul(p4[:, ns], lct_r[:],
                                             v_cur[:, ns], start=True,
                                             stop=True)
                        for n0 in (0, 512):
                            ns = slice(n0, n0 + 512)
                            nc.tensor.matmul(pw[:, ns], lms_r[:],
                                             v_cur[:, ns], start=True,
                                             stop=False)
                        for n0 in (0, 512):
                            ns = slice(n0, n0 + 512)
                            nc.tensor.matmul(pw[:, ns], lbf_r[:],
                                             D["t1"][:, ns], start=False,
                                             stop=False)
                        for n0 in (0, 512):
                            ns = slice(n0, n0 + 512)
                            nc.tensor.matmul(pw[:, ns], lcf_r[:],
                                             D["t2"][:, ns], start=False,
                                             stop=False)
                        # --- scalar drains (p3/p4 only; cs after absorb) ---
                        D["p3s"] = work.tile([128, CHUNK], BF16, tag=f"p3s{g}", name=f"p3s{g}")
                        D["p4s"] = work.tile([128, CHUNK], BF16, tag=f"p4s{g}", name=f"p4s{g}")
                        nc.scalar.activation(D["p3s"][:], p3[:], ACTF.Copy)
                        nc.scalar.activation(D["p4s"][:], p4[:], ACTF.Copy)
                    # --- combine: m1 (gps) / m2 (DVE) muls, PE-absorbed ---
                    for g in range(NCH):
                        S, D = st[g], T[g]
                        gsl = slice(g * CHUNK, (g + 1) * CHUNK)
                        m1 = work.tile([128, CHUNK], BF16, tag=f"m1{g}")
                        m2 = work.tile([128, CHUNK], BF16, tag=f"m2{g}")
                        nc.gpsimd.tensor_mul(m1[:], xtb[:, gsl], D["p3s"][:])
                        nc.vector.tensor_mul(m2[:], ytb[:, gsl], D["p4s"][:])
                        pw = D["pw"]
                        has_t4 = j > 0 and not last
                        for n0 in (0, 512):
                            ns = slice(n0, n0 + 512)
                            nc.tensor.matmul(pw[:, ns], ident_r[:],
                                             m1[:, ns], start=False,
                                             stop=False)
                        for n0 in (0, 512):
                            ns = slice(n0, n0 + 512)
                            nc.tensor.matmul(pw[:, ns], ident_r[:],
                                             m2[:, ns], start=False,
                                             stop=not has_t4)
                        if has_t4:
                            for n0 in (0, 512):
                                ns = slice(n0, n0 + 512)
                                nc.tensor.matmul(pw[:, ns], identn_r[:],
                                                 D["t4"][:, ns], start=False,
                                                 stop=n0 == 512)
                    # --- cs drain + alpha inner product + reduce ---
                    for g in range(NCH):
                        S, D = st[g], T[g]
                        cs_t = work.tile([128, CHUNK], F32, tag=f"cs{g}",
                                         name=f"cs{g}")
                        nc.scalar.activation(cs_t[:], D["pw"][:], ACTF.Copy)
                        D["c"] = cs_t
                        p_t = work.tile([128, CHUNK], BF16, tag=f"pq{g}")
                        nc.vector.tensor_mul(p_t[:], S["v"][:], cs_t[:])
                        D["p_t"] = p_t
                        ab = pbcp.tile([128, CHUNK], F32, tag="pbc")
                        for n0 in (0, 512):
                            ns = slice(n0, n0 + 512)
                            nc.tensor.matmul(ab[:, ns], obd_r[:],
                                             D["p_t"][:, ns], start=True,
                                             stop=True)
                        D["ab"] = ab
                    # --- rows + ortho + beta + normalize ---
                    for g in range(NCH):
                        S, D = st[g], T[g]
                        csl = slice(g * CHUNK, (g + 1) * CHUNK)
                        ab = D["ab"]
                        ra0 = work.tile([1, CHUNK], F32, tag=f"ra0{g}",
                                        name=f"ra0{g}")
                        ra1 = work.tile([1, CHUNK], F32, tag=f"ra1{g}",
                                        name=f"ra1{g}")
                        nc.scalar.activation(ra0[:], ab[0:1, :], ACTF.Copy)
                        nc.scalar.activation(ra1[:], ab[64:65, :], ACTF.Copy)
                        nc.sync.dma_start(out=stage_a[2 * j : 2 * j + 1, csl],
                                          in_=ra0[:])
                        nc.sync.dma_start(
                            out=stage_a[2 * j + 1 : 2 * j + 2, csl],
                            in_=ra1[:])
                        if not last:
                            t3 = work.tile([128, CHUNK], F32, tag=f"m1{g}")
                            nc.vector.tensor_mul(t3[:], ab[:], S["v"][:])
                            w = work.tile([128, CHUNK], F32, tag=f"m2{g}")
                            nc.vector.tensor_sub(w[:], D["c"][:], t3[:])
                            q_t = work.tile([128, CHUNK], BF16, tag=f"pq{g}")
                            nc.scalar.activation(q_t[:], w[:], ACTF.Square)
                            b2 = p3p.tile([128, CHUNK], F32, tag="p3")
                            for n0 in (0, 512):
                                ns = slice(n0, n0 + 512)
                                nc.tensor.matmul(b2[:, ns], obd_r[:],
                                                 q_t[:, ns], start=True,
                                                 stop=True)
                            bb = bbp.tile([128, CHUNK], F32, tag=f"bb{g}")
                            nc.scalar.activation(bb[:], b2[:], ACTF.Sqrt,
                                                 bias=epst[:], scale=1.0)
                            nc.sync.dma_start(
                                out=stage_b[2 * j : 2 * j + 1, csl],
                                in_=bb[0:1, :])
                            nc.sync.dma_start(
                                out=stage_b[2 * j + 1 : 2 * j + 2, csl],
                                in_=bb[64:65, :])
                            rb = work.tile([128, CHUNK], F32, tag=f"p3s{g}")
                            nc.vector.reciprocal_approx_fast(out=rb[:],
                                                             in_=bb[:])
                            v_nxt = vpool.tile([128, CHUNK], BF16,
                                               tag=f"v{g}")
                            nc.vector.tensor_mul(v_nxt[:], w[:], rb[:])
                            S["vp"] = S["v"]
                            S["v"] = v_nxt
                            S["bb"] = bb

            # ---------------- Phase 2: transpose + Sturm ----------------
            with (
                tc.tile_pool(name="bis", bufs=1) as bis,
                tc.tile_pool(name="st3", bufs=1) as st3,
                tc.tile_pool(name="ptp", bufs=1, space="PSUM") as ptp,
            ):
                ident = bis.tile([128, 128], F32)
                make_identity(nc, ident[:])

                pta = ptp.tile([128, TG * ROWS_A], F32, tag="pta")
                ptb = ptp.tile([128, TG * 64], F32, tag="ptb")
                for t in range(TG):
                    csl = slice(t * 128, (t + 1) * 128)
                    nc.tensor.transpose(pta[:, t * ROWS_A : (t + 1) * ROWS_A],
                                        stage_a[0:ROWS_A, csl],
                                        ident[0:ROWS_A, 0:ROWS_A])
                    nc.tensor.transpose(
                        ptb[:, t * 64 : t * 64 + ROWS_B],
                        stage_b[0:ROWS_B, csl], ident[0:ROWS_B, 0:ROWS_B])
                td_a = bis.tile([128, TG, ROWS_A], F32)
                td_b = bis.tile([128, TG, ROWS_B], F32)
                nc.vector.tensor_copy(td_a[:], pta[:])
                nc.vector.tensor_copy(td_b[:],
                                      _strided_free(ptb[:], 64, TG, ROWS_B))

                import concourse.bass as bass_mod

                def jdims_ap(tile_ap, nj, step0=2):
                    d = list(tile_ap.ap)
                    return bass_mod.AP(
                        tensor=tile_ap.tensor, offset=tile_ap.offset,
                        ap=[d[0], d[1], [1, 2], [step0, nj]],
                    )

                absb = td_b
                g = bis.tile([128, TG, ROWS_A], F32)
                nc.vector.tensor_copy(g[:], td_a[:])
                stt(g[:, :, 2:ROWS_A], g[:, :, 2:ROWS_A], absb[:],
                    OP.subtract)
                stt(g[:, :, 0:ROWS_B], g[:, :, 0:ROWS_B], absb[:],
                    OP.subtract)

                lo = bis.tile([128, TG, 2], F32)
                hi = bis.tile([128, TG, 2], F32)
                nc.vector.tensor_reduce(lo[:], jdims_ap(g[:], K),
                                        mybir.AxisListType.X, OP.min)
                if idx == 0:
                    nc.vector.tensor_reduce(hi[:], jdims_ap(td_a[:], K),
                                            mybir.AxisListType.X, OP.min)
                else:
                    g2 = g
                    nc.vector.tensor_copy(g2[:], td_a[:])
                    stt(g2[:, :, 2:ROWS_A], g2[:, :, 2:ROWS_A], absb[:],
                        OP.add)
                    stt(g2[:, :, 0:ROWS_B], g2[:, :, 0:ROWS_B], absb[:],
                        OP.add)
                    nc.vector.tensor_reduce(hi[:], jdims_ap(g2[:], K),
                                            mybir.AxisListType.X, OP.max)

                # j-major flat copies of the tridiagonal (stt needs <=3-dim APs)
                SEG = TG * 2
                td_a2 = bis.tile([128, K * SEG], F32)
                td_b2 = bis.tile([128, NB * SEG], F32)
                nc.vector.tensor_copy(td_a2[:], _jmajor(td_a[:], K, TG))
                nc.vector.tensor_copy(td_b2[:], _jmajor(td_b[:], NB, TG))
                stt(td_b2[:], td_b2[:], td_b2[:], OP.mult)

                cs = bis.tile([128, NS, TG, 2], F32)
                for s in range(NS):
                    nc.vector.memset(cs[:, s, :, :],
                                     float(s + 1) / float(NS + 1))

                sig = bis.tile([128, NS, TG, 2], F32)
                d_t = bis.tile([128, TG, 2], F32)
                pA = st3.tile([128, NS * SEG], F32, tag="pA")
                pB = st3.tile([128, NS * SEG], F32, tag="pB")
                pC = st3.tile([128, NS * SEG], F32, tag="pC")
                cA = st3.tile([128, NS, TG, 2], F32, tag="cA")
                cB = st3.tile([128, NS, TG, 2], F32, tag="cB")
                ca_t = st3.tile([128, NS * SEG], F32, tag="ca")
                u_t = st3.tile([128, NS * SEG], F32, tag="u")
                tb_t = st3.tile([128, NS * SEG], F32, tag="tb")
                sg_t = st3.tile([128, NS * SEG], F32, tag="sg")
                mle = bis.tile([128, TG, 2], I32)
                mgt = bis.tile([128, TG, 2], I32)

                thr = float(idx) + 0.5
                for ip in range(PASSES):
                    stt(d_t[:], hi[:], lo[:], OP.subtract)
                    stt(_flat(sig[:]), _flat(cs[:]),
                        _brow(_flat(d_t[:]), 0, SEG), OP.mult)
                    stt(_flat(sig[:]), _flat(sig[:]),
                        _brow(_flat(lo[:]), 0, SEG), OP.add)
                    po, pp, pn = pA, pB, pC
                    nc.vector.memset(po[:], 1.0)
                    stt(pp[:], _brow(td_a2[:], 0, SEG), _flat(sig[:]),
                        OP.subtract)
                    cnt, cnt_nxt = cA, cB
                    nc.vector.tensor_scalar(out=_flat(cnt[:]), in0=pp[:],
                                            scalar1=0.0, scalar2=None,
                                            op0=OP.is_lt)
                    for j in range(1, K):
                        stt(ca_t[:], _brow(td_a2[:], j, SEG), _flat(sig[:]),
                            OP.subtract)
                        stt(u_t[:], ca_t[:], pp[:], OP.mult)
                        stt(tb_t[:], _brow(td_b2[:], j - 1, SEG), po[:],
                            OP.mult)
                        stt(pn[:], u_t[:], tb_t[:], OP.subtract)
                        stt(sg_t[:], pn[:], pp[:], OP.mult)
                        nc.vector.scalar_tensor_tensor(
                            out=_flat(cnt_nxt[:]), in0=sg_t[:], scalar=0.0,
                            in1=_flat(cnt[:]), op0=OP.is_lt, op1=OP.add)
                        po, pp, pn = pp, pn, po
                        cnt, cnt_nxt = cnt_nxt, cnt
                    for s in range(NS):
                        nc.vector.tensor_scalar(out=mle[:],
                                                in0=cnt[:, s, :, :],
                                                scalar1=thr, scalar2=None,
                                                op0=OP.is_le)
                        nc.vector.copy_predicated(out=lo[:], mask=mle[:],
                                                  data=sig[:, s, :, :])
                    for s in range(NS - 1, -1, -1):
                        nc.vector.tensor_scalar(out=mgt[:],
                                                in0=cnt[:, s, :, :],
                                                scalar1=thr, scalar2=None,
                                                op0=OP.is_gt)
                        nc.vector.copy_predicated(out=hi[:], mask=mgt[:],
                                                  data=sig[:, s, :, :])

                lam_t = bis.tile([128, TG, 2], F32)
                stt(lam_t[:], lo[:], hi[:], OP.add)
                nc.vector.tensor_scalar(out=lam_t[:], in0=lam_t[:],
                                        scalar1=OUT_SCALE, scalar2=None,
                                        op0=OP.mult)
                lam_ap = lam_out.rearrange("(h t p) -> h p t", h=2, t=TG,
                                           p=128)
                for h in range(2):
                    nc.sync.dma_start(out=lam_ap[h], in_=lam_t[:, :, h])

    nc.compile()
    return nc


def kernel(x, y, A, B, C, eigval_idx):
    from concourse.bass_utils import run_bass_kernel_spmd

    idx = int(np.asarray(eigval_idx))
    nc = _program(idx)

    import ml_dtypes

    A32 = np.asarray(A, np.float32) * C_OP
    B32 = np.asarray(B, np.float32) * C_OP
    C32 = np.asarray(C, np.float32) * C_OP
    bf = ml_dtypes.bfloat16
    lms = _bd(A32 + A32.T).astype(bf)
    lbf = _bd(B32.T).astype(bf)
    lcf = _bd(C32.T).astype(bf)
    lbt = _bd(B32).astype(bf)
    lct = _bd(C32).astype(bf)
    obd = _bd(np.ones((64, 64), np.float32)).astype(bf)
    v0 = np.concatenate([_v0_vec(), _v0_vec()]).reshape(128, 1)

    xT = np.ascontiguousarray(np.asarray(x, np.float32).T)  # (64, BATCH)
    yT = np.ascontiguousarray(np.asarray(y, np.float32).T)

    in_maps = []
    for c in range(NCORES):
        b0 = c * SHARD
        xc = np.concatenate(
            [xT[:, b0 : b0 + NFREE], xT[:, b0 + NFREE : b0 + SHARD]], axis=0
        )
        yc = np.concatenate(
            [yT[:, b0 : b0 + NFREE], yT[:, b0 + NFREE : b0 + SHARD]], axis=0
        )
        in_maps.append(
            {
                "x": np.ascontiguousarray(xc),
                "y": np.ascontiguousarray(yc),
                "lms": lms, "lbf": lbf, "lcf": lcf, "lbt": lbt, "lct": lct,
                "obd": obd, "v0": v0,
            }
        )

    res = run_bass_kernel_spmd(nc, in_maps, core_ids=list(range(NCORES)))
    out = np.concatenate([res.results[c]["lam"] for c in range(NCORES)])
    return out.reshape(BATCH, 1).astype(np.float32)


# revision 18
# speedup vs baseline: 1.5837x; 1.1044x over previous
"""Bass/Trainium2 kernel for nn_BivariateSpectral: batched smallest-eigenvalue of
S_b = sym(A + B*diag(x_b) + C*diag(y_b)), b = 0..32767, each 64x64, 8 NeuronCores.

v2 (per core, data-parallel over batch):
  Phase 1 - batched Lanczos (K steps) on D_b = (M_b + M_b^T)/64 = S_b/32.
    D v = Ah v + Bh(x*v) + Ch(y*v) + x*(Bt v) + y*(Ct v); dim on partitions
    (two batch-halves as partitions 0-63/64-127, block-diag stationaries),
    batch on free dim.  PSUM is drained by the Scalar engine (activation
    copies) so the elementwise chain runs SBUF-only via scalar_tensor_tensor
    (2x_2p DVE mode).  alpha/beta^2 per column via ones-block-diag matmuls;
    beta-term folded into c before the alpha inner product (orthogonality).
    Tridiagonal rows staged to SBUF partitions 2j+h (no DRAM roundtrip).
  Phase 2 - PE-transpose the [2K, cols] stages into batch-on-partitions
    layout, then Sturm bisection (NS shifts x PASSES, division-free fp32
    char-poly recurrence).  Output scaled back by 32.
"""

import functools
import numpy as np

BATCH, DIM = 32768, 64
NCORES = 8
SHARD = BATCH // NCORES      # 4096 batch elems per core
NFREE = SHARD // 2           # 2048 free columns (two partition-halves)
CHUNK = 1024                 # columns per group
NCH = NFREE // CHUNK         # 2 groups
K = 32                       # Lanczos steps
NB = K - 1
ROWS_A = 2 * K               # 64 rows in stage_a (2j+h)
ROWS_B = 2 * NB              # 62 rows in stage_b
TG = NFREE // 128            # 16 transpose column-groups
NS = 6                       # bisection shifts per pass
PASSES = 3
C_OP = np.float32(1.0 / 64.0)   # A,B,C host prescale: D = (M+M^T)/64 = S/32
OUT_SCALE = 16.0                # lam_S = 32 * 0.5 * (lo+hi)


def _v0_vec():
    rng = np.random.default_rng(1234)
    v = rng.standard_normal(DIM).astype(np.float64)
    v /= np.sqrt((v * v).sum())
    return v.astype(np.float32)


def _bd(m):
    out = np.zeros((128, 128), np.float32)
    out[:64, :64] = m
    out[64:, 64:] = m
    return out


def _bcast_s(ap, extra_off=0, count=2, ns=None):
    """[128, T, R] AP -> [128, ns, T, count] with 0-step shift dim."""
    import concourse.bass as bass
    if ns is None:
        ns = NS
    dims = list(ap.ap)
    part = dims[0]
    tdim = dims[1]
    return bass.AP(
        tensor=ap.tensor,
        offset=ap.offset + extra_off,
        ap=[part, [0, ns], tdim, [1, count]],
    )


def _bcast_flat(ap, ns=None):
    """[128, T, 2] AP -> [128, ns, T, 2] via 0-step shift dim."""
    import concourse.bass as bass
    if ns is None:
        ns = NS
    dims = list(ap.ap)
    return bass.AP(tensor=ap.tensor, offset=ap.offset, ap=[dims[0], [0, ns]] + dims[1:])


def _two_rows(ap, stride=64, count=2):
    """AP over a [128, F] tile selecting partitions {0, stride}: -> [2, F]."""
    import concourse.bass as bass
    dims = list(ap.ap)
    return bass.AP(
        tensor=ap.tensor, offset=ap.offset, ap=[[stride, count]] + dims[1:]
    )


def _flat(ap):
    """Contiguous tile AP -> 2-dim [128, total_free] AP."""
    import concourse.bass as bass
    import numpy as np
    dims = list(ap.ap)
    total = 1
    for st, ct in dims[1:]:
        total *= ct
    return bass.AP(tensor=ap.tensor, offset=ap.offset,
                   ap=[dims[0], [1, total]])


def _brow(ap, j, seg, ns=None):
    """j-major flat tile AP -> [128, ns(bcast), seg] at row j."""
    import concourse.bass as bass
    if ns is None:
        ns = NS
    dims = list(ap.ap)
    return bass.AP(tensor=ap.tensor, offset=ap.offset + j * seg,
                   ap=[dims[0], [0, ns], [1, seg]])


def _jmajor(ap, nj, nt, step0=2, inner_stride=None):
    """[128, T, R] t-major AP -> 4-dim [128, nj, T, 2] j-major read AP."""
    import concourse.bass as bass
    dims = list(ap.ap)
    rstride = dims[2][0] if len(dims) > 2 else 1
    tstride = dims[1][0]
    return bass.AP(tensor=ap.tensor, offset=ap.offset,
                   ap=[dims[0], [step0 * rstride, nj], [tstride, nt],
                       [rstride, 2]])


def _strided_free(ap, stride, count, inner):
    """[128, F] tile AP -> [128, count, inner] with free stride for count dim."""
    import concourse.bass as bass
    dims = list(ap.ap)
    return bass.AP(
        tensor=ap.tensor, offset=ap.offset,
        ap=[dims[0], [stride, count], [1, inner]],
    )


@functools.lru_cache(maxsize=4)
def _program(idx: int):
    import concourse.bacc as bacc
    import concourse.bass as bass
    import concourse.mybir as mybir
    import concourse.tile as tile
    from concourse.masks import make_identity

    F32 = mybir.dt.float32
    F32R = mybir.dt.float32r
    BF16 = mybir.dt.bfloat16
    I32 = mybir.dt.int32
    OP = mybir.AluOpType
    ACTF = mybir.ActivationFunctionType

    nc = bacc.Bacc("TRN2", target_bir_lowering=False, debug=False)

    x_in = nc.dram_tensor("x", [128, NFREE], F32, kind="ExternalInput").ap()
    y_in = nc.dram_tensor("y", [128, NFREE], F32, kind="ExternalInput").ap()
    lms_in = nc.dram_tensor("lms", [128, 128], BF16, kind="ExternalInput").ap()
    lbf_in = nc.dram_tensor("lbf", [128, 128], BF16, kind="ExternalInput").ap()
    lcf_in = nc.dram_tensor("lcf", [128, 128], BF16, kind="ExternalInput").ap()
    lbt_in = nc.dram_tensor("lbt", [128, 128], BF16, kind="ExternalInput").ap()
    lct_in = nc.dram_tensor("lct", [128, 128], BF16, kind="ExternalInput").ap()
    obd_in = nc.dram_tensor("obd", [128, 128], BF16, kind="ExternalInput").ap()
    v0_in = nc.dram_tensor("v0", [128, 1], F32, kind="ExternalInput").ap()
    lam_out = nc.dram_tensor("lam", [SHARD], F32, kind="ExternalOutput").ap()

    def stt(out, in0, in1, op1, scalar=1.0, op0=OP.mult):
        nc.vector.scalar_tensor_tensor(out=out, in0=in0, scalar=scalar,
                                       in1=in1, op0=op0, op1=op1)

    with tile.TileContext(nc) as tc:
        with tc.tile_pool(name="stage", bufs=1) as stagep:
            stage_a = stagep.tile([128, NFREE], F32)
            stage_b = stagep.tile([128, NFREE], F32)

            # ---------------- Phase 1: Lanczos ----------------
            with (
                tc.tile_pool(name="singles", bufs=1) as singles,
                tc.tile_pool(name="vpool", bufs=3) as vpool,
                tc.tile_pool(name="work", bufs=1) as work,
                tc.tile_pool(name="bbp", bufs=2) as bbp,
                tc.tile_pool(name="p3p", bufs=1, space="PSUM") as p3p,
                tc.tile_pool(name="p4p", bufs=1, space="PSUM") as p4p,
                tc.tile_pool(name="pwp", bufs=1, space="PSUM") as pwp,
                tc.tile_pool(name="pbc", bufs=1, space="PSUM") as pbcp,
            ):
                xt = singles.tile([128, NFREE], F32)
                yt = singles.tile([128, NFREE], F32)
                nc.sync.dma_start(out=xt[:], in_=x_in)
                nc.sync.dma_start(out=yt[:], in_=y_in)
                xtb = singles.tile([128, NFREE], BF16)
                ytb = singles.tile([128, NFREE], BF16)
                nc.vector.tensor_copy(xtb[:], xt[:])
                nc.vector.tensor_copy(ytb[:], yt[:])
                mats = {}
                for nm, src in (("lms", lms_in), ("lbf", lbf_in),
                                ("lcf", lcf_in), ("lbt", lbt_in),
                                ("lct", lct_in), ("obd", obd_in)):
                    t_ = singles.tile([128, 128], BF16, name=f"m_{nm}")
                    nc.sync.dma_start(out=t_[:], in_=src)
                    mats[nm] = t_
                lms_r, lbf_r, lcf_r = mats["lms"], mats["lbf"], mats["lcf"]
                lbt_r, lct_r, obd_r = mats["lbt"], mats["lct"], mats["obd"]
                identt = singles.tile([128, 128], F32)
                make_identity(nc, identt[:])
                ident_r = singles.tile([128, 128], BF16)
                nc.vector.tensor_copy(ident_r[:], identt[:])
                identn_r = singles.tile([128, 128], BF16)
                nc.scalar.activation(identn_r[:], identt[:], ACTF.Copy,
                                     scale=-1.0)
                v0t = singles.tile([128, 1], F32)
                nc.sync.dma_start(out=v0t[:], in_=v0_in)
                epst = singles.tile([128, 1], F32)
                nc.vector.memset(epst[:], 1e-12)

                st = []
                for g in range(NCH):
                    v_cur = vpool.tile([128, CHUNK], BF16, tag=f"v{g}")
                    nc.vector.tensor_copy(
                        v_cur[:], v0t[:, 0:1].to_broadcast((128, CHUNK)))
                    st.append({"v": v_cur, "vp": None, "bb": None})

                for j in range(K):
                    last = j == K - 1
                    T = [{} for _ in range(NCH)]
                    # --- front muls: t4 first (earliest deps) then t1/t2 ---
                    for g in range(NCH):
                        S, D = st[g], T[g]
                        if j > 0 and not last:
                            D["t4"] = work.tile([128, CHUNK], BF16,
                                                tag=f"t4{g}", name=f"t4{g}")
                            nc.gpsimd.tensor_mul(D["t4"][:], S["bb"][:],
                                                 S["vp"][:])
                    for g in range(NCH):
                        S, D = st[g], T[g]
                        gsl = slice(g * CHUNK, (g + 1) * CHUNK)
                        D["t1"] = work.tile([128, CHUNK], BF16, tag=f"t1{g}", name=f"t1{g}")
                        D["t2"] = work.tile([128, CHUNK], BF16, tag=f"t2{g}", name=f"t2{g}")
                        nc.gpsimd.tensor_mul(D["t1"][:], xtb[:, gsl], S["v"][:])
                        nc.vector.tensor_mul(D["t2"][:], ytb[:, gsl], S["v"][:])
                    # --- matmuls, grouped by stationary within a group ---
                    for g in range(NCH):
                        S, D = st[g], T[g]
                        v_cur = S["v"]
                        p3 = p3p.tile([128, CHUNK], F32, tag="p3")
                        p4 = p4p.tile([128, CHUNK], F32, tag="p4")
                        pw = pwp.tile([128, CHUNK], F32, tag="pw")
                        D["p3"], D["p4"], D["pw"] = p3, p4, pw
                        for n0 in (0, 512):
                            ns = slice(n0, n0 + 512)
                            nc.tensor.matmul(p3[:, ns], lbt_r[:],
                                             v_cur[:, ns], start=True,
                                             stop=True)
                        for n0 in (0, 512):
                            ns = slice(n0, n0 + 512)
                            nc.tensor.matmul(p4[:, ns], lct_r[:],
                                             v_cur[:, ns], start=True,
                                             stop=True)
                        for n0 in (0, 512):
                            ns = slice(n0, n0 + 512)
                            nc.tensor.matmul(pw[:, ns], lms_r[:],
                                             v_cur[:, ns], start=True,
                                             stop=False)
                        for n0 in (0, 512):
                            ns = slice(n0, n0 + 512)
                            nc.tensor.matmul(pw[:, ns], lbf_r[:],
                                             D["t1"][:, ns], start=False,
                                             stop=False)
                        for n0 in (0, 512):
                            ns = slice(n0, n0 + 512)
                            nc.tensor.matmul(pw[:, ns], lcf_r[:],
                                             D["t2"][:, ns], start=False,
                                             stop=False)
                        # --- scalar drains (p3/p4 only; cs after absorb) ---
                        D["p3s"] = work.tile([128, CHUNK], BF16, tag=f"p3s{g}", name=f"p3s{g}")
                        D["p4s"] = work.tile([128, CHUNK], BF16, tag=f"p4s{g}", name=f"p4s{g}")
                        nc.scalar.activation(D["p3s"][:], p3[:], ACTF.Copy)
                        nc.scalar.activation(D["p4s"][:], p4[:], ACTF.Copy)
                    # --- combine: m1 (gps) / m2 (DVE) muls, PE-absorbed ---
                    for g in range(NCH):
                        S, D = st[g], T[g]
                        gsl = slice(g * CHUNK, (g + 1) * CHUNK)
                        m1 = work.tile([128, CHUNK], BF16, tag=f"m1{g}")
                        m2 = work.tile([128, CHUNK], BF16, tag=f"m2{g}")
                        nc.gpsimd.tensor_mul(m1[:], xtb[:, gsl], D["p3s"][:])
                        nc.vector.tensor_mul(m2[:], ytb[:, gsl], D["p4s"][:])
                        pw = D["pw"]
                        has_t4 = j > 0 and not last
                        for n0 in (0, 512):
                            ns = slice(n0, n0 + 512)
                            nc.tensor.matmul(pw[:, ns], ident_r[:],
                                             m1[:, ns], start=False,
                                             stop=False)
                        for n0 in (0, 512):
                            ns = slice(n0, n0 + 512)
                            nc.tensor.matmul(pw[:, ns], ident_r[:],
                                             m2[:, ns], start=False,
                                             stop=not has_t4)
                        if has_t4:
                            for n0 in (0, 512):
                                ns = slice(n0, n0 + 512)
                                nc.tensor.matmul(pw[:, ns], identn_r[:],
                                                 D["t4"][:, ns], start=False,
                                                 stop=n0 == 512)
                        cs_t = work.tile([128, CHUNK], F32, tag=f"cs{g}",
                                         name=f"cs{g}")
                        nc.scalar.activation(cs_t[:], pw[:], ACTF.Copy)
                        D["c"] = cs_t
                        p_t = work.tile([128, CHUNK], BF16, tag=f"pq{g}")
                        nc.vector.tensor_mul(p_t[:], S["v"][:], cs_t[:])
                        D["p_t"] = p_t
                    # --- alpha reduce + rows + ortho + beta + normalize ---
                    for g in range(NCH):
                        S, D = st[g], T[g]
                        csl = slice(g * CHUNK, (g + 1) * CHUNK)
                        ab = pbcp.tile([128, CHUNK], F32, tag="pbc")
                        for n0 in (0, 512):
                            ns = slice(n0, n0 + 512)
                            nc.tensor.matmul(ab[:, ns], obd_r[:],
                                             D["p_t"][:, ns], start=True,
                                             stop=True)
                        ab_s = work.tile([128, CHUNK], F32, tag=f"abs{g}",
                                         name=f"abs{g}")
                        nc.scalar.activation(ab_s[:], ab[:], ACTF.Copy)
                        nc.sync.dma_start(out=stage_a[2 * j : 2 * j + 1, csl],
                                          in_=ab_s[0:1, :])
                        nc.sync.dma_start(
                            out=stage_a[2 * j + 1 : 2 * j + 2, csl],
                            in_=ab_s[64:65, :])
                        if not last:
                            t3 = work.tile([128, CHUNK], F32, tag=f"m1{g}")
                            nc.vector.tensor_mul(t3[:], ab_s[:], S["v"][:])
                            w = work.tile([128, CHUNK], F32, tag=f"m2{g}")
                            nc.vector.tensor_sub(w[:], D["c"][:], t3[:])
                            q_t = work.tile([128, CHUNK], BF16, tag=f"pq{g}")
                            nc.scalar.activation(q_t[:], w[:], ACTF.Square)
                            b2 = p3p.tile([128, CHUNK], F32, tag="p3")
                            for n0 in (0, 512):
                                ns = slice(n0, n0 + 512)
                                nc.tensor.matmul(b2[:, ns], obd_r[:],
                                                 q_t[:, ns], start=True,
                                                 stop=True)
                            bb = bbp.tile([128, CHUNK], F32, tag=f"bb{g}")
                            nc.scalar.activation(bb[:], b2[:], ACTF.Sqrt,
                                                 bias=epst[:], scale=1.0)
                            nc.sync.dma_start(
                                out=stage_b[2 * j : 2 * j + 1, csl],
                                in_=bb[0:1, :])
                            nc.sync.dma_start(
                                out=stage_b[2 * j + 1 : 2 * j + 2, csl],
                                in_=bb[64:65, :])
                            rb = work.tile([128, CHUNK], F32, tag=f"p3s{g}")
                            nc.vector.reciprocal_approx_fast(out=rb[:],
                                                             in_=bb[:])
                            v_nxt = vpool.tile([128, CHUNK], BF16,
                                               tag=f"v{g}")
                            nc.vector.tensor_mul(v_nxt[:], w[:], rb[:])
                            S["vp"] = S["v"]
                            S["v"] = v_nxt
                            S["bb"] = bb

            # ---------------- Phase 2: transpose + Sturm ----------------
            with (
                tc.tile_pool(name="bis", bufs=1) as bis,
                tc.tile_pool(name="st3", bufs=1) as st3,
                tc.tile_pool(name="ptp", bufs=1, space="PSUM") as ptp,
            ):
                ident = bis.tile([128, 128], F32)
                make_identity(nc, ident[:])

                pta = ptp.tile([128, TG * ROWS_A], F32, tag="pta")
                ptb = ptp.tile([128, TG * 64], F32, tag="ptb")
                for t in range(TG):
                    csl = slice(t * 128, (t + 1) * 128)
                    nc.tensor.transpose(pta[:, t * ROWS_A : (t + 1) * ROWS_A],
                                        stage_a[0:ROWS_A, csl],
                                        ident[0:ROWS_A, 0:ROWS_A])
                    nc.tensor.transpose(
                        ptb[:, t * 64 : t * 64 + ROWS_B],
                        stage_b[0:ROWS_B, csl], ident[0:ROWS_B, 0:ROWS_B])
                td_a = bis.tile([128, TG, ROWS_A], F32)
                td_b = bis.tile([128, TG, ROWS_B], F32)
                nc.vector.tensor_copy(td_a[:], pta[:])
                nc.vector.tensor_copy(td_b[:],
                                      _strided_free(ptb[:], 64, TG, ROWS_B))

                import concourse.bass as bass_mod

                def jdims_ap(tile_ap, nj, step0=2):
                    d = list(tile_ap.ap)
                    return bass_mod.AP(
                        tensor=tile_ap.tensor, offset=tile_ap.offset,
                        ap=[d[0], d[1], [1, 2], [step0, nj]],
                    )

                absb = td_b
                g = bis.tile([128, TG, ROWS_A], F32)
                nc.vector.tensor_copy(g[:], td_a[:])
                stt(g[:, :, 2:ROWS_A], g[:, :, 2:ROWS_A], absb[:],
                    OP.subtract)
                stt(g[:, :, 0:ROWS_B], g[:, :, 0:ROWS_B], absb[:],
                    OP.subtract)

                lo = bis.tile([128, TG, 2], F32)
                hi = bis.tile([128, TG, 2], F32)
                nc.vector.tensor_reduce(lo[:], jdims_ap(g[:], K),
                                        mybir.AxisListType.X, OP.min)
                if idx == 0:
                    nc.vector.tensor_reduce(hi[:], jdims_ap(td_a[:], K),
                                            mybir.AxisListType.X, OP.min)
                else:
                    g2 = g
                    nc.vector.tensor_copy(g2[:], td_a[:])
                    stt(g2[:, :, 2:ROWS_A], g2[:, :, 2:ROWS_A], absb[:],
                        OP.add)
                    stt(g2[:, :, 0:ROWS_B], g2[:, :, 0:ROWS_B], absb[:],
                        OP.add)
                    nc.vector.tensor_reduce(hi[:], jdims_ap(g2[:], K),
                                            mybir.AxisListType.X, OP.max)

                # j-major flat copies of the tridiagonal (stt needs <=3-dim APs)
                SEG = TG * 2
                td_a2 = bis.tile([128, K * SEG], F32)
                td_b2 = bis.tile([128, NB * SEG], F32)
                nc.vector.tensor_copy(td_a2[:], _jmajor(td_a[:], K, TG))
                nc.vector.tensor_copy(td_b2[:], _jmajor(td_b[:], NB, TG))
                stt(td_b2[:], td_b2[:], td_b2[:], OP.mult)

                cs = bis.tile([128, NS, TG, 2], F32)
                for s in range(NS):
                    nc.vector.memset(cs[:, s, :, :],
                                     float(s + 1) / float(NS + 1))

                sig = bis.tile([128, NS, TG, 2], F32)
                d_t = bis.tile([128, TG, 2], F32)
                pA = st3.tile([128, NS * SEG], F32, tag="pA")
                pB = st3.tile([128, NS * SEG], F32, tag="pB")
                pC = st3.tile([128, NS * SEG], F32, tag="pC")
                cA = st3.tile([128, NS, TG, 2], F32, tag="cA")
                cB = st3.tile([128, NS, TG, 2], F32, tag="cB")
                ca_t = st3.tile([128, NS * SEG], F32, tag="ca")
                u_t = st3.tile([128, NS * SEG], F32, tag="u")
                tb_t = st3.tile([128, NS * SEG], F32, tag="tb")
                sg_t = st3.tile([128, NS * SEG], F32, tag="sg")
                mle = bis.tile([128, TG, 2], I32)
                mgt = bis.tile([128, TG, 2], I32)

                thr = float(idx) + 0.5
                for ip in range(PASSES):
                    stt(d_t[:], hi[:], lo[:], OP.subtract)
                    stt(_flat(sig[:]), _flat(cs[:]),
                        _brow(_flat(d_t[:]), 0, SEG), OP.mult)
                    stt(_flat(sig[:]), _flat(sig[:]),
                        _brow(_flat(lo[:]), 0, SEG), OP.add)
                    po, pp, pn = pA, pB, pC
                    nc.vector.memset(po[:], 1.0)
                    stt(pp[:], _brow(td_a2[:], 0, SEG), _flat(sig[:]),
                        OP.subtract)
                    cnt, cnt_nxt = cA, cB
                    nc.vector.tensor_scalar(out=_flat(cnt[:]), in0=pp[:],
                                            scalar1=0.0, scalar2=None,
                                            op0=OP.is_lt)
                    for j in range(1, K):
                        stt(ca_t[:], _brow(td_a2[:], j, SEG), _flat(sig[:]),
                            OP.subtract)
                        stt(u_t[:], ca_t[:], pp[:], OP.mult)
                        stt(tb_t[:], _brow(td_b2[:], j - 1, SEG), po[:],
                            OP.mult)
                        stt(pn[:], u_t[:], tb_t[:], OP.subtract)
                        stt(sg_t[:], pn[:], pp[:], OP.mult)
                        nc.vector.scalar_tensor_tensor(
                            out=_flat(cnt_nxt[:]), in0=sg_t[:], scalar=0.0,
                            in1=_flat(cnt[:]), op0=OP.is_lt, op1=OP.add)
                        po, pp, pn = pp, pn, po
                        cnt, cnt_nxt = cnt_nxt, cnt
                    for s in range(NS):
                        nc.vector.tensor_scalar(out=mle[:],
                                                in0=cnt[:, s, :, :],
                                                scalar1=thr, scalar2=None,
                                                op0=OP.is_le)
                        nc.vector.copy_predicated(out=lo[:], mask=mle[:],
                                                  data=sig[:, s, :, :])
                    for s in range(NS - 1, -1, -1):
                        nc.vector.tensor_scalar(out=mgt[:],
                                                in0=cnt[:, s, :, :],
                                                scalar1=thr, scalar2=None,
                                                op0=OP.is_gt)
                        nc.vector.copy_predicated(out=hi[:], mask=mgt[:],
                                                  data=sig[:, s, :, :])

                lam_t = bis.tile([128, TG, 2], F32)
                stt(lam_t[:], lo[:], hi[:], OP.add)
                nc.vector.tensor_scalar(out=lam_t[:], in0=lam_t[:],
                                        scalar1=OUT_SCALE, scalar2=None,
                                        op0=OP.mult)
                lam_ap = lam_out.rearrange("(h t p) -> h p t", h=2, t=TG,
                                           p=128)
                for h in range(2):
                    nc.sync.dma_start(out=lam_ap[h], in_=lam_t[:, :, h])

    nc.compile()
    return nc


def kernel(x, y, A, B, C, eigval_idx):
    from concourse.bass_utils import run_bass_kernel_spmd

    idx = int(np.asarray(eigval_idx))
    nc = _program(idx)

    import ml_dtypes

    A32 = np.asarray(A, np.float32) * C_OP
    B32 = np.asarray(B, np.float32) * C_OP
    C32 = np.asarray(C, np.float32) * C_OP
    bf = ml_dtypes.bfloat16
    lms = _bd(A32 + A32.T).astype(bf)
    lbf = _bd(B32.T).astype(bf)
    lcf = _bd(C32.T).astype(bf)
    lbt = _bd(B32).astype(bf)
    lct = _bd(C32).astype(bf)
    obd = _bd(np.ones((64, 64), np.float32)).astype(bf)
    v0 = np.concatenate([_v0_vec(), _v0_vec()]).reshape(128, 1)

    xT = np.ascontiguousarray(np.asarray(x, np.float32).T)  # (64, BATCH)
    yT = np.ascontiguousarray(np.asarray(y, np.float32).T)

    in_maps = []
    for c in range(NCORES):
        b0 = c * SHARD
        xc = np.concatenate(
            [xT[:, b0 : b0 + NFREE], xT[:, b0 + NFREE : b0 + SHARD]], axis=0
        )
        yc = np.concatenate(
            [yT[:, b0 : b0 + NFREE], yT[:, b0 + NFREE : b0 + SHARD]], axis=0
        )
        in_maps.append(
            {
                "x": np.ascontiguousarray(xc),
                "y": np.ascontiguousarray(yc),
                "lms": lms, "lbf": lbf, "lcf": lcf, "lbt": lbt, "lct": lct,
                "obd": obd, "v0": v0,
            }
        )

    res = run_bass_kernel_spmd(nc, in_maps, core_ids=list(range(NCORES)))
    out = np.concatenate([res.results[c]["lam"] for c in range(NCORES)])
    return out.reshape(BATCH, 1).astype(np.float32)


# revision 19
# speedup vs baseline: 1.6305x; 1.0296x over previous
"""Bass/Trainium2 kernel for nn_BivariateSpectral: batched smallest-eigenvalue of
S_b = sym(A + B*diag(x_b) + C*diag(y_b)), b = 0..32767, each 64x64, 8 NeuronCores.

v2 (per core, data-parallel over batch):
  Phase 1 - batched Lanczos (K steps) on D_b = (M_b + M_b^T)/64 = S_b/32.
    D v = Ah v + Bh(x*v) + Ch(y*v) + x*(Bt v) + y*(Ct v); dim on partitions
    (two batch-halves as partitions 0-63/64-127, block-diag stationaries),
    batch on free dim.  PSUM is drained by the Scalar engine (activation
    copies) so the elementwise chain runs SBUF-only via scalar_tensor_tensor
    (2x_2p DVE mode).  alpha/beta^2 per column via ones-block-diag matmuls;
    beta-term folded into c before the alpha inner product (orthogonality).
    Tridiagonal rows staged to SBUF partitions 2j+h (no DRAM roundtrip).
  Phase 2 - PE-transpose the [2K, cols] stages into batch-on-partitions
    layout, then Sturm bisection (NS shifts x PASSES, division-free fp32
    char-poly recurrence).  Output scaled back by 32.
"""

import functools
import numpy as np

BATCH, DIM = 32768, 64
NCORES = 8
SHARD = BATCH // NCORES      # 4096 batch elems per core
NFREE = SHARD // 2           # 2048 free columns (two partition-halves)
CHUNK = 1024                 # columns per group
NCH = NFREE // CHUNK         # 2 groups
K = 32                       # Lanczos steps
NB = K - 1
ROWS_A = 2 * K               # 64 rows in stage_a (2j+h)
ROWS_B = 2 * NB              # 62 rows in stage_b
TG = NFREE // 128            # 16 transpose column-groups
NS = 8                       # bisection shifts per pass
PASSES = 2
C_OP = np.float32(1.0 / 64.0)   # A,B,C host prescale: D = (M+M^T)/64 = S/32
OUT_SCALE = 16.0                # lam_S = 32 * 0.5 * (lo+hi)


def _v0_vec():
    rng = np.random.default_rng(1234)
    v = rng.standard_normal(DIM).astype(np.float64)
    v /= np.sqrt((v * v).sum())
    return v.astype(np.float32)


def _bd(m):
    out = np.zeros((128, 128), np.float32)
    out[:64, :64] = m
    out[64:, 64:] = m
    return out


def _bcast_s(ap, extra_off=0, count=2, ns=None):
    """[128, T, R] AP -> [128, ns, T, count] with 0-step shift dim."""
    import concourse.bass as bass
    if ns is None:
        ns = NS
    dims = list(ap.ap)
    part = dims[0]
    tdim = dims[1]
    return bass.AP(
        tensor=ap.tensor,
        offset=ap.offset + extra_off,
        ap=[part, [0, ns], tdim, [1, count]],
    )


def _bcast_flat(ap, ns=None):
    """[128, T, 2] AP -> [128, ns, T, 2] via 0-step shift dim."""
    import concourse.bass as bass
    if ns is None:
        ns = NS
    dims = list(ap.ap)
    return bass.AP(tensor=ap.tensor, offset=ap.offset, ap=[dims[0], [0, ns]] + dims[1:])


def _two_rows(ap, stride=64, count=2):
    """AP over a [128, F] tile selecting partitions {0, stride}: -> [2, F]."""
    import concourse.bass as bass
    dims = list(ap.ap)
    return bass.AP(
        tensor=ap.tensor, offset=ap.offset, ap=[[stride, count]] + dims[1:]
    )


def _flat(ap):
    """Contiguous tile AP -> 2-dim [128, total_free] AP."""
    import concourse.bass as bass
    import numpy as np
    dims = list(ap.ap)
    total = 1
    for st, ct in dims[1:]:
        total *= ct
    return bass.AP(tensor=ap.tensor, offset=ap.offset,
                   ap=[dims[0], [1, total]])


def _brow(ap, j, seg, ns=None):
    """j-major flat tile AP -> [128, ns(bcast), seg] at row j."""
    import concourse.bass as bass
    if ns is None:
        ns = NS
    dims = list(ap.ap)
    return bass.AP(tensor=ap.tensor, offset=ap.offset + j * seg,
                   ap=[dims[0], [0, ns], [1, seg]])


def _jmajor(ap, nj, nt, step0=2, inner_stride=None):
    """[128, T, R] t-major AP -> 4-dim [128, nj, T, 2] j-major read AP."""
    import concourse.bass as bass
    dims = list(ap.ap)
    rstride = dims[2][0] if len(dims) > 2 else 1
    tstride = dims[1][0]
    return bass.AP(tensor=ap.tensor, offset=ap.offset,
                   ap=[dims[0], [step0 * rstride, nj], [tstride, nt],
                       [rstride, 2]])


def _strided_free(ap, stride, count, inner):
    """[128, F] tile AP -> [128, count, inner] with free stride for count dim."""
    import concourse.bass as bass
    dims = list(ap.ap)
    return bass.AP(
        tensor=ap.tensor, offset=ap.offset,
        ap=[dims[0], [stride, count], [1, inner]],
    )


@functools.lru_cache(maxsize=4)
def _program(idx: int):
    import concourse.bacc as bacc
    import concourse.bass as bass
    import concourse.mybir as mybir
    import concourse.tile as tile
    from concourse.masks import make_identity

    F32 = mybir.dt.float32
    F32R = mybir.dt.float32r
    BF16 = mybir.dt.bfloat16
    I32 = mybir.dt.int32
    OP = mybir.AluOpType
    ACTF = mybir.ActivationFunctionType

    nc = bacc.Bacc("TRN2", target_bir_lowering=False, debug=False)

    x_in = nc.dram_tensor("x", [128, NFREE], F32, kind="ExternalInput").ap()
    y_in = nc.dram_tensor("y", [128, NFREE], F32, kind="ExternalInput").ap()
    lms_in = nc.dram_tensor("lms", [128, 128], BF16, kind="ExternalInput").ap()
    lbf_in = nc.dram_tensor("lbf", [128, 128], BF16, kind="ExternalInput").ap()
    lcf_in = nc.dram_tensor("lcf", [128, 128], BF16, kind="ExternalInput").ap()
    lbt_in = nc.dram_tensor("lbt", [128, 128], BF16, kind="ExternalInput").ap()
    lct_in = nc.dram_tensor("lct", [128, 128], BF16, kind="ExternalInput").ap()
    obd_in = nc.dram_tensor("obd", [128, 128], BF16, kind="ExternalInput").ap()
    v0_in = nc.dram_tensor("v0", [128, 1], F32, kind="ExternalInput").ap()
    lam_out = nc.dram_tensor("lam", [SHARD], F32, kind="ExternalOutput").ap()

    def stt(out, in0, in1, op1, scalar=1.0, op0=OP.mult):
        nc.vector.scalar_tensor_tensor(out=out, in0=in0, scalar=scalar,
                                       in1=in1, op0=op0, op1=op1)

    with tile.TileContext(nc) as tc:
        with tc.tile_pool(name="stage", bufs=1) as stagep:
            stage_a = stagep.tile([128, NFREE], F32)
            stage_b = stagep.tile([128, NFREE], F32)

            # ---------------- Phase 1: Lanczos ----------------
            with (
                tc.tile_pool(name="singles", bufs=1) as singles,
                tc.tile_pool(name="vpool", bufs=3) as vpool,
                tc.tile_pool(name="work", bufs=1) as work,
                tc.tile_pool(name="bbp", bufs=2) as bbp,
                tc.tile_pool(name="p3p", bufs=1, space="PSUM") as p3p,
                tc.tile_pool(name="p4p", bufs=1, space="PSUM") as p4p,
                tc.tile_pool(name="pwp", bufs=1, space="PSUM") as pwp,
                tc.tile_pool(name="pbc", bufs=1, space="PSUM") as pbcp,
            ):
                xt = singles.tile([128, NFREE], F32)
                yt = singles.tile([128, NFREE], F32)
                nc.sync.dma_start(out=xt[:], in_=x_in)
                nc.sync.dma_start(out=yt[:], in_=y_in)
                xtb = singles.tile([128, NFREE], BF16)
                ytb = singles.tile([128, NFREE], BF16)
                nc.vector.tensor_copy(xtb[:], xt[:])
                nc.vector.tensor_copy(ytb[:], yt[:])
                mats = {}
                for nm, src in (("lms", lms_in), ("lbf", lbf_in),
                                ("lcf", lcf_in), ("lbt", lbt_in),
                                ("lct", lct_in), ("obd", obd_in)):
                    t_ = singles.tile([128, 128], BF16, name=f"m_{nm}")
                    nc.sync.dma_start(out=t_[:], in_=src)
                    mats[nm] = t_
                lms_r, lbf_r, lcf_r = mats["lms"], mats["lbf"], mats["lcf"]
                lbt_r, lct_r, obd_r = mats["lbt"], mats["lct"], mats["obd"]
                identt = singles.tile([128, 128], F32)
                make_identity(nc, identt[:])
                ident_r = singles.tile([128, 128], BF16)
                nc.vector.tensor_copy(ident_r[:], identt[:])
                identn_r = singles.tile([128, 128], BF16)
                nc.scalar.activation(identn_r[:], identt[:], ACTF.Copy,
                                     scale=-1.0)
                v0t = singles.tile([128, 1], F32)
                nc.sync.dma_start(out=v0t[:], in_=v0_in)
                epst = singles.tile([128, 1], F32)
                nc.vector.memset(epst[:], 1e-12)

                st = []
                for g in range(NCH):
                    v_cur = vpool.tile([128, CHUNK], BF16, tag=f"v{g}")
                    nc.vector.tensor_copy(
                        v_cur[:], v0t[:, 0:1].to_broadcast((128, CHUNK)))
                    st.append({"v": v_cur, "vp": None, "bb": None})

                for j in range(K):
                    last = j == K - 1
                    T = [{} for _ in range(NCH)]
                    # --- front muls: t4 first (earliest deps) then t1/t2 ---
                    for g in range(NCH):
                        S, D = st[g], T[g]
                        if j > 0 and not last:
                            D["t4"] = work.tile([128, CHUNK], BF16,
                                                tag=f"t4{g}", name=f"t4{g}")
                            nc.gpsimd.tensor_mul(D["t4"][:], S["bb"][:],
                                                 S["vp"][:])
                    for g in range(NCH):
                        S, D = st[g], T[g]
                        gsl = slice(g * CHUNK, (g + 1) * CHUNK)
                        D["t1"] = work.tile([128, CHUNK], BF16, tag=f"t1{g}", name=f"t1{g}")
                        D["t2"] = work.tile([128, CHUNK], BF16, tag=f"t2{g}", name=f"t2{g}")
                        nc.gpsimd.tensor_mul(D["t1"][:], xtb[:, gsl], S["v"][:])
                        nc.vector.tensor_mul(D["t2"][:], ytb[:, gsl], S["v"][:])
                    # --- matmuls, grouped by stationary within a group ---
                    for g in range(NCH):
                        S, D = st[g], T[g]
                        v_cur = S["v"]
                        p3 = p3p.tile([128, CHUNK], F32, tag="p3")
                        p4 = p4p.tile([128, CHUNK], F32, tag="p4")
                        pw = pwp.tile([128, CHUNK], F32, tag="pw")
                        D["p3"], D["p4"], D["pw"] = p3, p4, pw
                        for n0 in (0, 512):
                            ns = slice(n0, n0 + 512)
                            nc.tensor.matmul(p3[:, ns], lbt_r[:],
                                             v_cur[:, ns], start=True,
                                             stop=True)
                        for n0 in (0, 512):
                            ns = slice(n0, n0 + 512)
                            nc.tensor.matmul(p4[:, ns], lct_r[:],
                                             v_cur[:, ns], start=True,
                                             stop=True)
                        for n0 in (0, 512):
                            ns = slice(n0, n0 + 512)
                            nc.tensor.matmul(pw[:, ns], lms_r[:],
                                             v_cur[:, ns], start=True,
                                             stop=False)
                        for n0 in (0, 512):
                            ns = slice(n0, n0 + 512)
                            nc.tensor.matmul(pw[:, ns], lbf_r[:],
                                             D["t1"][:, ns], start=False,
                                             stop=False)
                        for n0 in (0, 512):
                            ns = slice(n0, n0 + 512)
                            nc.tensor.matmul(pw[:, ns], lcf_r[:],
                                             D["t2"][:, ns], start=False,
                                             stop=False)
                        # --- scalar drains (p3/p4 only; cs after absorb) ---
                        D["p3s"] = work.tile([128, CHUNK], BF16, tag=f"p3s{g}", name=f"p3s{g}")
                        D["p4s"] = work.tile([128, CHUNK], BF16, tag=f"p4s{g}", name=f"p4s{g}")
                        nc.scalar.activation(D["p3s"][:], p3[:], ACTF.Copy)
                        nc.scalar.activation(D["p4s"][:], p4[:], ACTF.Copy)
                    # --- combine: m1 (gps) / m2 (DVE) muls, PE-absorbed ---
                    for g in range(NCH):
                        S, D = st[g], T[g]
                        gsl = slice(g * CHUNK, (g + 1) * CHUNK)
                        m1 = work.tile([128, CHUNK], BF16, tag=f"m1{g}")
                        m2 = work.tile([128, CHUNK], BF16, tag=f"m2{g}")
                        nc.gpsimd.tensor_mul(m1[:], xtb[:, gsl], D["p3s"][:])
                        nc.vector.tensor_mul(m2[:], ytb[:, gsl], D["p4s"][:])
                        pw = D["pw"]
                        has_t4 = j > 0 and not last
                        for n0 in (0, 512):
                            ns = slice(n0, n0 + 512)
                            nc.tensor.matmul(pw[:, ns], ident_r[:],
                                             m1[:, ns], start=False,
                                             stop=False)
                        for n0 in (0, 512):
                            ns = slice(n0, n0 + 512)
                            nc.tensor.matmul(pw[:, ns], ident_r[:],
                                             m2[:, ns], start=False,
                                             stop=not has_t4)
                        if has_t4:
                            for n0 in (0, 512):
                                ns = slice(n0, n0 + 512)
                                nc.tensor.matmul(pw[:, ns], identn_r[:],
                                                 D["t4"][:, ns], start=False,
                                                 stop=n0 == 512)
                        cs_t = work.tile([128, CHUNK], F32, tag=f"cs{g}",
                                         name=f"cs{g}")
                        nc.scalar.activation(cs_t[:], pw[:], ACTF.Copy)
                        D["c"] = cs_t
                        p_t = work.tile([128, CHUNK], BF16, tag=f"pq{g}")
                        nc.vector.tensor_mul(p_t[:], S["v"][:], cs_t[:])
                        D["p_t"] = p_t
                    # --- alpha reduce + rows + ortho + beta + normalize ---
                    for g in range(NCH):
                        S, D = st[g], T[g]
                        csl = slice(g * CHUNK, (g + 1) * CHUNK)
                        ab = pbcp.tile([128, CHUNK], F32, tag="pbc")
                        for n0 in (0, 512):
                            ns = slice(n0, n0 + 512)
                            nc.tensor.matmul(ab[:, ns], obd_r[:],
                                             D["p_t"][:, ns], start=True,
                                             stop=True)
                        ab_s = work.tile([128, CHUNK], F32, tag=f"abs{g}",
                                         name=f"abs{g}")
                        nc.scalar.activation(ab_s[:], ab[:], ACTF.Copy)
                        nc.sync.dma_start(out=stage_a[2 * j : 2 * j + 1, csl],
                                          in_=ab_s[0:1, :])
                        nc.sync.dma_start(
                            out=stage_a[2 * j + 1 : 2 * j + 2, csl],
                            in_=ab_s[64:65, :])
                        if not last:
                            t3 = work.tile([128, CHUNK], F32, tag=f"m1{g}")
                            nc.vector.tensor_mul(t3[:], ab_s[:], S["v"][:])
                            w = work.tile([128, CHUNK], F32, tag=f"m2{g}")
                            nc.vector.tensor_sub(w[:], D["c"][:], t3[:])
                            q_t = work.tile([128, CHUNK], BF16, tag=f"pq{g}")
                            nc.scalar.activation(q_t[:], w[:], ACTF.Square)
                            b2 = p3p.tile([128, CHUNK], F32, tag="p3")
                            for n0 in (0, 512):
                                ns = slice(n0, n0 + 512)
                                nc.tensor.matmul(b2[:, ns], obd_r[:],
                                                 q_t[:, ns], start=True,
                                                 stop=True)
                            bb = bbp.tile([128, CHUNK], F32, tag=f"bb{g}")
                            nc.scalar.activation(bb[:], b2[:], ACTF.Sqrt,
                                                 bias=epst[:], scale=1.0)
                            nc.sync.dma_start(
                                out=stage_b[2 * j : 2 * j + 1, csl],
                                in_=bb[0:1, :])
                            nc.sync.dma_start(
                                out=stage_b[2 * j + 1 : 2 * j + 2, csl],
                                in_=bb[64:65, :])
                            rb = work.tile([128, CHUNK], F32, tag=f"p3s{g}")
                            nc.vector.reciprocal_approx_fast(out=rb[:],
                                                             in_=bb[:])
                            v_nxt = vpool.tile([128, CHUNK], BF16,
                                               tag=f"v{g}")
                            nc.vector.tensor_mul(v_nxt[:], w[:], rb[:])
                            S["vp"] = S["v"]
                            S["v"] = v_nxt
                            S["bb"] = bb

            # ---------------- Phase 2: transpose + Sturm ----------------
            with (
                tc.tile_pool(name="bis", bufs=1) as bis,
                tc.tile_pool(name="st3", bufs=1) as st3,
                tc.tile_pool(name="ptp", bufs=1, space="PSUM") as ptp,
            ):
                ident = bis.tile([128, 128], F32)
                make_identity(nc, ident[:])

                pta = ptp.tile([128, TG * ROWS_A], F32, tag="pta")
                ptb = ptp.tile([128, TG * 64], F32, tag="ptb")
                for t in range(TG):
                    csl = slice(t * 128, (t + 1) * 128)
                    nc.tensor.transpose(pta[:, t * ROWS_A : (t + 1) * ROWS_A],
                                        stage_a[0:ROWS_A, csl],
                                        ident[0:ROWS_A, 0:ROWS_A])
                    nc.tensor.transpose(
                        ptb[:, t * 64 : t * 64 + ROWS_B],
                        stage_b[0:ROWS_B, csl], ident[0:ROWS_B, 0:ROWS_B])
                td_a = bis.tile([128, TG, ROWS_A], F32)
                td_b = bis.tile([128, TG, ROWS_B], F32)
                nc.vector.tensor_copy(td_a[:], pta[:])
                nc.vector.tensor_copy(td_b[:],
                                      _strided_free(ptb[:], 64, TG, ROWS_B))

                import concourse.bass as bass_mod

                def jdims_ap(tile_ap, nj, step0=2):
                    d = list(tile_ap.ap)
                    return bass_mod.AP(
                        tensor=tile_ap.tensor, offset=tile_ap.offset,
                        ap=[d[0], d[1], [1, 2], [step0, nj]],
                    )

                absb = td_b
                g = bis.tile([128, TG, ROWS_A], F32)
                nc.vector.tensor_copy(g[:], td_a[:])
                stt(g[:, :, 2:ROWS_A], g[:, :, 2:ROWS_A], absb[:],
                    OP.subtract)
                stt(g[:, :, 0:ROWS_B], g[:, :, 0:ROWS_B], absb[:],
                    OP.subtract)

                lo = bis.tile([128, TG, 2], F32)
                hi = bis.tile([128, TG, 2], F32)
                nc.vector.tensor_reduce(lo[:], jdims_ap(g[:], K),
                                        mybir.AxisListType.X, OP.min)
                if idx == 0:
                    nc.vector.tensor_reduce(hi[:], jdims_ap(td_a[:], K),
                                            mybir.AxisListType.X, OP.min)
                else:
                    g2 = g
                    nc.vector.tensor_copy(g2[:], td_a[:])
                    stt(g2[:, :, 2:ROWS_A], g2[:, :, 2:ROWS_A], absb[:],
                        OP.add)
                    stt(g2[:, :, 0:ROWS_B], g2[:, :, 0:ROWS_B], absb[:],
                        OP.add)
                    nc.vector.tensor_reduce(hi[:], jdims_ap(g2[:], K),
                                            mybir.AxisListType.X, OP.max)

                # j-major flat copies of the tridiagonal (stt needs <=3-dim APs)
                SEG = TG * 2
                td_a2 = bis.tile([128, K * SEG], F32)
                td_b2 = bis.tile([128, NB * SEG], F32)
                nc.vector.tensor_copy(td_a2[:], _jmajor(td_a[:], K, TG))
                nc.vector.tensor_copy(td_b2[:], _jmajor(td_b[:], NB, TG))
                stt(td_b2[:], td_b2[:], td_b2[:], OP.mult)

                cs = bis.tile([128, NS, TG, 2], F32)
                for s in range(NS):
                    nc.vector.memset(cs[:, s, :, :],
                                     float(s + 1) / float(NS + 1))

                sig = bis.tile([128, NS, TG, 2], F32)
                d_t = bis.tile([128, TG, 2], F32)
                pA = st3.tile([128, NS * SEG], F32, tag="pA")
                pB = st3.tile([128, NS * SEG], F32, tag="pB")
                pC = st3.tile([128, NS * SEG], F32, tag="pC")
                cA = st3.tile([128, NS, TG, 2], F32, tag="cA")
                cB = st3.tile([128, NS, TG, 2], F32, tag="cB")
                ca_t = st3.tile([128, NS * SEG], F32, tag="ca")
                u_t = st3.tile([128, NS * SEG], F32, tag="u")
                tb_t = st3.tile([128, NS * SEG], F32, tag="tb")
                sg_t = st3.tile([128, NS * SEG], F32, tag="sg")
                mle = bis.tile([128, TG, 2], I32)
                mgt = bis.tile([128, TG, 2], I32)

                thr = float(idx) + 0.5
                for ip in range(PASSES):
                    stt(d_t[:], hi[:], lo[:], OP.subtract)
                    stt(_flat(sig[:]), _flat(cs[:]),
                        _brow(_flat(d_t[:]), 0, SEG), OP.mult)
                    stt(_flat(sig[:]), _flat(sig[:]),
                        _brow(_flat(lo[:]), 0, SEG), OP.add)
                    po, pp, pn = pA, pB, pC
                    nc.vector.memset(po[:], 1.0)
                    stt(pp[:], _brow(td_a2[:], 0, SEG), _flat(sig[:]),
                        OP.subtract)
                    cnt, cnt_nxt = cA, cB
                    nc.vector.tensor_scalar(out=_flat(cnt[:]), in0=pp[:],
                                            scalar1=0.0, scalar2=None,
                                            op0=OP.is_lt)
                    for j in range(1, K):
                        stt(ca_t[:], _brow(td_a2[:], j, SEG), _flat(sig[:]),
                            OP.subtract)
                        stt(u_t[:], ca_t[:], pp[:], OP.mult)
                        stt(tb_t[:], _brow(td_b2[:], j - 1, SEG), po[:],
                            OP.mult)
                        stt(pn[:], u_t[:], tb_t[:], OP.subtract)
                        stt(sg_t[:], pn[:], pp[:], OP.mult)
                        nc.vector.scalar_tensor_tensor(
                            out=_flat(cnt_nxt[:]), in0=sg_t[:], scalar=0.0,
                            in1=_flat(cnt[:]), op0=OP.is_lt, op1=OP.add)
                        po, pp, pn = pp, pn, po
                        cnt, cnt_nxt = cnt_nxt, cnt
                    for s in range(NS):
                        nc.vector.tensor_scalar(out=mle[:],
                                                in0=cnt[:, s, :, :],
                                                scalar1=thr, scalar2=None,
                                                op0=OP.is_le)
                        nc.vector.copy_predicated(out=lo[:], mask=mle[:],
                                                  data=sig[:, s, :, :])
                    for s in range(NS - 1, -1, -1):
                        nc.vector.tensor_scalar(out=mgt[:],
                                                in0=cnt[:, s, :, :],
                                                scalar1=thr, scalar2=None,
                                                op0=OP.is_gt)
                        nc.vector.copy_predicated(out=hi[:], mask=mgt[:],
                                                  data=sig[:, s, :, :])

                lam_t = bis.tile([128, TG, 2], F32)
                stt(lam_t[:], lo[:], hi[:], OP.add)
                nc.vector.tensor_scalar(out=lam_t[:], in0=lam_t[:],
                                        scalar1=OUT_SCALE, scalar2=None,
                                        op0=OP.mult)
                lam_ap = lam_out.rearrange("(h t p) -> h p t", h=2, t=TG,
                                           p=128)
                for h in range(2):
                    nc.sync.dma_start(out=lam_ap[h], in_=lam_t[:, :, h])

    nc.compile()
    return nc


def kernel(x, y, A, B, C, eigval_idx):
    from concourse.bass_utils import run_bass_kernel_spmd

    idx = int(np.asarray(eigval_idx))
    nc = _program(idx)

    import ml_dtypes

    A32 = np.asarray(A, np.float32) * C_OP
    B32 = np.asarray(B, np.float32) * C_OP
    C32 = np.asarray(C, np.float32) * C_OP
    bf = ml_dtypes.bfloat16
    lms = _bd(A32 + A32.T).astype(bf)
    lbf = _bd(B32.T).astype(bf)
    lcf = _bd(C32.T).astype(bf)
    lbt = _bd(B32).astype(bf)
    lct = _bd(C32).astype(bf)
    obd = _bd(np.ones((64, 64), np.float32)).astype(bf)
    v0 = np.concatenate([_v0_vec(), _v0_vec()]).reshape(128, 1)

    xT = np.ascontiguousarray(np.asarray(x, np.float32).T)  # (64, BATCH)
    yT = np.ascontiguousarray(np.asarray(y, np.float32).T)

    in_maps = []
    for c in range(NCORES):
        b0 = c * SHARD
        xc = np.concatenate(
            [xT[:, b0 : b0 + NFREE], xT[:, b0 + NFREE : b0 + SHARD]], axis=0
        )
        yc = np.concatenate(
            [yT[:, b0 : b0 + NFREE], yT[:, b0 + NFREE : b0 + SHARD]], axis=0
        )
        in_maps.append(
            {
                "x": np.ascontiguousarray(xc),
                "y": np.ascontiguousarray(yc),
                "lms": lms, "lbf": lbf, "lcf": lcf, "lbt": lbt, "lct": lct,
                "obd": obd, "v0": v0,
            }
        )

    res = run_bass_kernel_spmd(nc, in_maps, core_ids=list(range(NCORES)))
    out = np.concatenate([res.results[c]["lam"] for c in range(NCORES)])
    return out.reshape(BATCH, 1).astype(np.float32)
